# revision 1
# baseline (speedup 1.0000x reference)
"""Trainium2 Bass kernel for nn_MultiHeadAttention (channel-attention transformer block).

Math (per batch b, with X* = reshape(*, [C, P]), P = 4096, C = 128, D = 512):
  Q = Xq @ (Wq/temp)^T, K = Xk @ Wk^T, V = Xv @ Wv^T            [C, D]
  per head h (8 heads, ld=64): A_h = softmax(Q_h K_h^T); O_h = A_h V_h
  O = silu(O); O = (O - mean)/(unbiased_std + eps)   (LN affine folded into fc)
  out_pre = (v + Wfc@ln_beta) + O @ (Wfc*ln_gamma)^T
  out = BatchNorm2d(out_pre)   (batch stats over (b,h,w), biased var)

Sharding: data-parallel over batch, 2 batches per core on 8 cores; BatchNorm
statistics combined with a tiny AllReduce ([128,2] per core).

Matmul dtype: float32r (fp32 bits, full PE rate at N>=256); BASS_MM_MODE can
switch to bf16 or plain f32. All inputs are host-packed so every DMA is a
fully contiguous transfer on both DRAM and SBUF sides.
"""

import os

import numpy as np

import concourse.mybir as mybir
import concourse.tile as tile
from concourse import bacc
from concourse.bass_utils import run_bass_kernel_spmd
from concourse.masks import make_identity

# ---- problem constants (hardcoded per contract) ----
B, C, HH, WW = 16, 128, 64, 64
P = HH * WW           # 4096
NH, LD = 8, 64
D = NH * LD           # 512
N_CORES = 8
BPC = B // N_CORES    # 2 batches per core
NPC = P // 512        # 8 quad-chunks over contraction / output tiles
LN_EPS = 1e-6
BN_EPS = 1e-5
F32 = mybir.dt.float32
F32R = mybir.dt.float32r
BF16 = mybir.dt.bfloat16

MODE = os.environ.get("BASS_MM_MODE", "f32r")  # f32r | bf16 | f32

_BUILD_CACHE: dict = {}
LAST_RESULTS = None  # BassKernelResults of the most recent run (for profiling)


def _emit(ctx, nc, tc, io):
    act_dt = {"f32r": F32R, "bf16": BF16, "f32": F32}[MODE]
    AF = mybir.ActivationFunctionType
    ALU = mybir.AluOpType
    AX = mybir.AxisListType

    def raw(ap):
        # f32 view of an f32r AP for DVE reads (pure byte copy, no re-round)
        return ap.bitcast(F32) if MODE == "f32r" else ap

    consts = ctx.enter_context(tc.tile_pool(name="consts", bufs=1))
    wpool = ctx.enter_context(tc.tile_pool(name="wpool", bufs=2))
    fcpool = ctx.enter_context(tc.tile_pool(name="fcpool", bufs=5))
    apool = ctx.enter_context(tc.tile_pool(name="apool", bufs=2))
    big = ctx.enter_context(tc.tile_pool(name="big", bufs=1))
    sb = ctx.enter_context(tc.tile_pool(name="sb", bufs=2))
    small = ctx.enter_context(tc.tile_pool(name="small", bufs=4))
    stat = ctx.enter_context(tc.tile_pool(name="stat", bufs=1))
    dram = ctx.enter_context(tc.tile_pool(name="dram", bufs=1, space="DRAM"))

    # identity for PE transposes (made in f32, cast to the matmul dtype);
    # a dummy transpose primes PE's view of the identity writer so later
    # transposes carry a single sync wait (HW allows 1 per instruction)
    ident_f = consts.tile([128, 128], F32, tag="identf", name="identf")
    make_identity(nc, ident_f)
    if MODE == "f32":
        ident = ident_f
    else:
        ident = consts.tile([128, 128], act_dt, tag="ident", name="ident")
        nc.vector.tensor_copy(out=ident, in_=ident_f)

    bng = consts.tile([128, 1], F32, tag="bng", name="bng")
    bnb = consts.tile([128, 1], F32, tag="bnb", name="bnb")
    nc.gpsimd.dma_start(out=bng, in_=io["bng"][:, :])
    nc.gpsimd.dma_start(out=bnb, in_=io["bnb"][:, :])

    out_sb = []
    for b in range(BPC):
        t = big.tile([128, P], F32, tag=f"veff{b}", name=f"veff{b}")
        nc.gpsimd.dma_start(out=t, in_=io["veff"][b, :, :])
        out_sb.append(t)

    # ---- phase A: QKV projections, accumulating over the P=4096 contraction ----
    ps_proj = ctx_a = tc.tile_pool(name="ps_proj", bufs=1, space="PSUM")
    ps_proj = ctx_a.__enter__()
    warm = ps_proj.tile([128, 128], act_dt, tag="warm", name="warm")
    nc.tensor.transpose(warm[:, :], ident[:, :], ident[:, :])
    Qp = [ps_proj.tile([128, D], F32, tag=f"Qp{b}", name=f"Qp{b}") for b in range(BPC)]
    Kp = [ps_proj.tile([128, D], F32, tag=f"Kp{b}", name=f"Kp{b}") for b in range(BPC)]
    Vp = [ps_proj.tile([128, D], F32, tag=f"Vp{b}", name=f"Vp{b}") for b in range(BPC)]

    for pc in range(NPC):
        wq_c = wpool.tile([128, 4, D], act_dt, tag="wq_c", name="wq_c")
        wk_c = wpool.tile([128, 4, D], act_dt, tag="wk_c", name="wk_c")
        wv_c = wpool.tile([128, 4, D], act_dt, tag="wv_c", name="wv_c")
        nc.sync.dma_start(out=wq_c, in_=io["wq"][pc])
        nc.scalar.dma_start(out=wk_c, in_=io["wk"][pc])
        nc.gpsimd.dma_start(out=wv_c, in_=io["wv"][pc])
        qcs, kcs, vcs = [], [], []
        for b in range(BPC):
            qc = apool.tile([128, 4, 128], act_dt, tag=f"qc{b}", name=f"qc{b}")
            kc = apool.tile([128, 4, 128], act_dt, tag=f"kc{b}", name=f"kc{b}")
            vc = apool.tile([128, 4, 128], act_dt, tag=f"vc{b}", name=f"vc{b}")
            nc.sync.dma_start(out=qc, in_=io["qT"][b, pc])
            nc.scalar.dma_start(out=kc, in_=io["kT"][b, pc])
            nc.gpsimd.dma_start(out=vc, in_=io["vT"][b, pc])
            qcs.append(qc); kcs.append(kc); vcs.append(vc)
        for j in range(4):
            st = pc == 0 and j == 0
            sp = pc == NPC - 1 and j == 3
            for b in range(BPC):
                nc.tensor.matmul(Qp[b][:, :], qcs[b][:, j, :], wq_c[:, j, :], start=st, stop=sp)
                nc.tensor.matmul(Kp[b][:, :], kcs[b][:, j, :], wk_c[:, j, :], start=st, stop=sp)
                nc.tensor.matmul(Vp[b][:, :], vcs[b][:, j, :], wv_c[:, j, :], start=st, stop=sp)

    # prefetch fc weights early (no data deps; sync queue is idle after phase A)
    wfcts = []
    for pt in range(NPC):
        wfct = fcpool.tile([128, 4, 512], act_dt, tag="wfct", name="wfct")
        nc.sync.dma_start(out=wfct, in_=io["wfc"][pt])
        wfcts.append(wfct)

    # ---- evacuate PSUM: copies for both batches free all 6 proj banks ----
    qkv_sb = []
    for b in range(BPC):
        Q_sb = sb.tile([128, D], act_dt, tag="Q_sb", name="Q_sb")
        K_sb = sb.tile([128, D], act_dt, tag="K_sb", name="K_sb")
        V_sb = sb.tile([128, D], act_dt, tag="V_sb", name="V_sb")
        nc.vector.tensor_copy(out=Q_sb, in_=Qp[b][:, :])
        nc.scalar.copy(out=K_sb, in_=Kp[b][:, :])
        nc.vector.tensor_copy(out=V_sb, in_=Vp[b][:, :])
        qkv_sb.append((Q_sb, K_sb, V_sb))
    ctx_a.__exit__(None, None, None)
    ps_s = ctx.enter_context(tc.tile_pool(name="ps_s", bufs=2, space="PSUM"))
    ps_o = ctx.enter_context(tc.tile_pool(name="ps_o", bufs=2, space="PSUM"))
    ps_fc = ctx.enter_context(tc.tile_pool(name="ps_fc", bufs=2, space="PSUM"))

    # per-channel partial sums: cols 0..15 = sum(out) per (b,pt), 16..31 = sum(out^2)
    pcols = stat.tile([128, 32], F32, tag="pcols", name="pcols")

    # ---- phases B-D per batch: attention, silu+LN ----
    xTs = []
    for b in range(BPC):
        Q_sb, K_sb, V_sb = qkv_sb[b]

        QT_sb = sb.tile([128, D], act_dt, tag="QT_sb", name="QT_sb")
        KT_sb = sb.tile([128, D], act_dt, tag="KT_sb", name="KT_sb")
        for src, dst in ((Q_sb, QT_sb), (K_sb, KT_sb)):
            for dc in range(4):
                tp = ps_s.tile([128, 128], act_dt, tag="stp", name="stp")
                nc.tensor.transpose(tp[:, :], src[:, dc * 128:(dc + 1) * 128], ident[:, :])
                nc.vector.tensor_copy(out=dst[:, dc * 128:(dc + 1) * 128], in_=raw(tp[:, :]))

        Opsum = ps_o.tile([128, D], F32, tag="O", name="O")
        Osc = sb.tile([128, D], F32, tag="Osc", name="Osc")
        for h in range(NH):
            po = (h % 2) * 64
            fo = (h // 2) * 128
            S = ps_s.tile([128, 128], F32, tag="S", name="S")
            nc.tensor.matmul(S[:, :], QT_sb[po:po + 64, fo:fo + 128],
                             KT_sb[po:po + 64, fo:fo + 128], start=True, stop=True)
            e_f = sb.tile([128, 128], F32, tag="e_f", name="e_f")
            lsum = small.tile([128, 1], F32, tag="lsum", name="lsum")
            nc.scalar.activation(out=e_f, in_=S[:, :], func=AF.Exp, accum_out=lsum)
            rs = small.tile([128, 1], F32, tag="rs", name="rs")
            nc.vector.reciprocal(rs, lsum)
            tpa = ps_s.tile([128, 128], F32, tag="stp", name="stp")
            nc.tensor.transpose(tpa[:, :], e_f[:, :], ident_f[:, :])
            aT = sb.tile([128, 128], act_dt, tag="aT", name="aT")
            nc.scalar.copy(out=aT, in_=tpa[:, :])
            nc.tensor.matmul(Opsum[:, h * 64:(h + 1) * 64], aT[:, :],
                             V_sb[:, h * 64:(h + 1) * 64], start=True, stop=True)
            nc.vector.tensor_scalar_mul(out=Osc[:, h * 64:(h + 1) * 64],
                                        in0=Opsum[:, h * 64:(h + 1) * 64],
                                        scalar1=rs)

        # silu + layernorm (affine folded into fc weights on host)
        sg = sb.tile([128, D], F32, tag="sg", name="sg")
        nc.scalar.activation(out=sg, in_=Osc, func=AF.Sigmoid)
        Osw = sb.tile([128, D], F32, tag="Osw", name="Osw")
        nc.vector.tensor_mul(out=Osw, in0=Osc, in1=sg)
        st6 = small.tile([128, 6], F32, tag="st6", name="st6")
        nc.vector.bn_stats(out=st6, in_=Osw)
        mv = small.tile([128, 2], F32, tag="mv", name="mv")
        nc.vector.bn_aggr(out=mv, in_=st6)
        sd = small.tile([128, 1], F32, tag="sd", name="sd")
        nc.scalar.activation(out=sd, in_=mv[:, 1:2], func=AF.Sqrt, scale=float(D) / (D - 1))
        nc.vector.tensor_scalar_add(out=sd, in0=sd, scalar1=LN_EPS)
        rstd = small.tile([128, 1], F32, tag="rstd", name="rstd")
        nc.vector.reciprocal(rstd, sd)
        xhat = sb.tile([128, D], act_dt, tag="xhat", name="xhat")
        nc.vector.tensor_scalar(out=xhat, in0=Osw, scalar1=mv[:, 0:1], scalar2=rstd,
                                op0=ALU.subtract, op1=ALU.mult)
        xT = sb.tile([128, D], act_dt, tag="xT", name="xT")
        for dc in range(4):
            tp = ps_s.tile([128, 128], act_dt, tag="stp", name="stp")
            nc.tensor.transpose(tp[:, :], xhat[:, dc * 128:(dc + 1) * 128], ident[:, :])
            nc.vector.tensor_copy(out=xT[:, dc * 128:(dc + 1) * 128], in_=raw(tp[:, :]))
        xTs.append(xT)

    # ---- phase D2: fc + residual + BN partial sums, streaming wfc ----
    for pt in range(NPC):
        for b in range(BPC):
            O2 = ps_fc.tile([128, 512], F32, tag="O2", name="O2")
            for dc in range(4):
                nc.tensor.matmul(O2[:, :], xTs[b][:, dc * 128:(dc + 1) * 128],
                                 wfcts[pt][:, dc, :], start=dc == 0, stop=dc == 3)
            seg = out_sb[b][:, pt * 512:(pt + 1) * 512]
            nc.vector.tensor_add(out=seg, in0=seg, in1=O2[:, :])
            nc.vector.reduce_sum(pcols[:, b * NPC + pt:b * NPC + pt + 1], seg, axis=AX.X)
            junk = sb.tile([128, 512], F32, tag="junk", name="junk")
            nc.scalar.activation(out=junk, in_=seg, func=AF.Square,
                                 accum_out=pcols[:, 16 + b * NPC + pt:17 + b * NPC + pt])

    # ---- phase E: BN stats AllReduce + normalize + store ----
    stats2 = stat.tile([128, 2], F32, tag="stats2", name="stats2")
    nc.vector.reduce_sum(stats2[:, 0:1], pcols[:, 0:16], axis=AX.X)
    nc.vector.reduce_sum(stats2[:, 1:2], pcols[:, 16:32], axis=AX.X)

    cin = dram.tile([128, 2], F32, tag="cin", name="cin")
    cout = dram.tile([128, 2], F32, tag="cout", name="cout")
    nc.gpsimd.dma_start(out=cin[:, :], in_=stats2)
    if os.environ.get("BASS_SKIP_COLL", "0") == "1":
        nc.gpsimd.dma_start(out=cout[:, :], in_=cin[:, :])
    else:
        nc.gpsimd.collective_compute(
            "AllReduce",
            ALU.add,
            replica_groups=[list(range(N_CORES))],
            ins=[cin.opt()],
            outs=[cout.opt()],
        )
    red = stat.tile([128, 2], F32, tag="red", name="red")
    nc.gpsimd.dma_start(out=red[:, :], in_=cout[:, :])

    inv_n = 1.0 / float(B * P)
    mean = small.tile([128, 1], F32, tag="mean", name="mean")
    nc.scalar.mul(out=mean, in_=red[:, 0:1], mul=inv_n)
    ex2 = small.tile([128, 1], F32, tag="ex2", name="ex2")
    nc.scalar.mul(out=ex2, in_=red[:, 1:2], mul=inv_n)
    msq = small.tile([128, 1], F32, tag="msq", name="msq")
    nc.vector.tensor_mul(out=msq, in0=mean, in1=mean)
    var = small.tile([128, 1], F32, tag="var", name="var")
    nc.vector.tensor_sub(out=var, in0=ex2, in1=msq)
    epsbn = consts.tile([128, 1], F32, tag="epsbn", name="epsbn")
    nc.vector.memset(epsbn, BN_EPS)
    sdv = small.tile([128, 1], F32, tag="sdv", name="sdv")
    nc.scalar.activation(out=sdv, in_=var, func=AF.Sqrt, bias=epsbn)
    invs = small.tile([128, 1], F32, tag="invs", name="invs")
    nc.vector.reciprocal(invs, sdv)
    scl = small.tile([128, 1], F32, tag="scl", name="scl")
    nc.vector.tensor_mul(out=scl, in0=bng, in1=invs)
    tmp = small.tile([128, 1], F32, tag="tmp", name="tmp")
    nc.vector.tensor_mul(out=tmp, in0=mean, in1=scl)
    shf = small.tile([128, 1], F32, tag="shf", name="shf")
    nc.vector.tensor_sub(out=shf, in0=bnb, in1=tmp)

    for b in range(BPC):
        for pt in range(NPC):
            seg = out_sb[b][:, pt * 512:(pt + 1) * 512]
            nc.vector.tensor_scalar(out=seg, in0=seg, scalar1=scl, scalar2=shf,
                                    op0=ALU.mult, op1=ALU.add)
            nc.gpsimd.dma_start(out=io["out"][b, :, pt * 512:(pt + 1) * 512], in_=seg)


def _build():
    key = (MODE, os.environ.get("BASS_SKIP_COLL", "0"))
    if key in _BUILD_CACHE:
        return _BUILD_CACHE[key]
    act_np = {"f32r": F32R, "bf16": BF16, "f32": F32}[MODE]
    nc = bacc.Bacc("TRN2", target_bir_lowering=False, debug=False, num_devices=N_CORES)
    io = {
        "qT": nc.dram_tensor("qT", [BPC, NPC, 128, 4, 128], act_np, kind="ExternalInput").ap(),
        "kT": nc.dram_tensor("kT", [BPC, NPC, 128, 4, 128], act_np, kind="ExternalInput").ap(),
        "vT": nc.dram_tensor("vT", [BPC, NPC, 128, 4, 128], act_np, kind="ExternalInput").ap(),
        "veff": nc.dram_tensor("veff", [BPC, C, P], F32, kind="ExternalInput").ap(),
        "wq": nc.dram_tensor("wq", [NPC, 128, 4, D], act_np, kind="ExternalInput").ap(),
        "wk": nc.dram_tensor("wk", [NPC, 128, 4, D], act_np, kind="ExternalInput").ap(),
        "wv": nc.dram_tensor("wv", [NPC, 128, 4, D], act_np, kind="ExternalInput").ap(),
        "wfc": nc.dram_tensor("wfc", [NPC, 128, 4, 512], act_np, kind="ExternalInput").ap(),
        "bng": nc.dram_tensor("bng", [C, 1], F32, kind="ExternalInput").ap(),
        "bnb": nc.dram_tensor("bnb", [C, 1], F32, kind="ExternalInput").ap(),
        "out": nc.dram_tensor("out", [BPC, C, P], F32, kind="ExternalOutput").ap(),
    }
    from contextlib import ExitStack
    with tile.TileContext(nc) as tc, ExitStack() as ctx:
        _emit(ctx, nc, tc, io)
    nc.compile()
    _BUILD_CACHE[key] = nc
    return nc


def _np_cast(x):
    if MODE == "bf16":
        import ml_dtypes
        return np.ascontiguousarray(np.asarray(x, np.float32).astype(ml_dtypes.bfloat16))
    return np.ascontiguousarray(np.asarray(x, np.float32))


def _pack_acts(xT):
    # [b, 4096, 128] -> [b, NPC, 128, 4, 128]  (pc-chunk, partition, j, c)
    b = xT.shape[0]
    return np.ascontiguousarray(
        xT.reshape(b, NPC, 4, 128, 128).transpose(0, 1, 3, 2, 4))


def _pack_w(w):
    # [4096, D] -> [NPC, 128, 4, D]
    return np.ascontiguousarray(w.reshape(NPC, 4, 128, -1).transpose(0, 2, 1, 3))


def kernel(v, k, q, w_qs, w_ks, w_vs, w_fc, ln_gamma, ln_beta, temperature,
           bn_gamma, bn_beta, **_ignored):
    v = np.asarray(v, np.float32)
    k = np.asarray(k, np.float32)
    q = np.asarray(q, np.float32)
    w_qs = np.asarray(w_qs, np.float32)
    w_ks = np.asarray(w_ks, np.float32)
    w_vs = np.asarray(w_vs, np.float32)
    w_fc = np.asarray(w_fc, np.float32)
    ln_gamma = np.asarray(ln_gamma, np.float32)
    ln_beta = np.asarray(ln_beta, np.float32)
    temp = float(np.asarray(temperature))
    bn_gamma = np.asarray(bn_gamma, np.float32)
    bn_beta = np.asarray(bn_beta, np.float32)

    qf = q.reshape(B, C, P)
    kf = k.reshape(B, C, P)
    vf = v.reshape(B, C, P)
    qT = _np_cast(_pack_acts(qf.transpose(0, 2, 1)))
    kT = _np_cast(_pack_acts(kf.transpose(0, 2, 1)))
    vT = _np_cast(_pack_acts(vf.transpose(0, 2, 1)))
    wq = _np_cast(_pack_w((w_qs / temp).T))
    wk = _np_cast(_pack_w(w_ks.T))
    wv = _np_cast(_pack_w(w_vs.T))
    # wfc packed as [pt, p, dc, c]: wfcT_eff[dc*128+p, pt*512+c]
    wfcT_eff = (w_fc * ln_gamma[None, :]).T  # [D, P]
    wfc = _np_cast(wfcT_eff.reshape(4, 128, NPC, 512).transpose(2, 1, 0, 3))
    bias_fc = (w_fc @ ln_beta).astype(np.float32)
    veff = np.ascontiguousarray(vf + bias_fc[None, None, :])
    bng = np.ascontiguousarray(bn_gamma.reshape(C, 1))
    bnb = np.ascontiguousarray(bn_beta.reshape(C, 1))

    nc = _build()
    in_maps = []
    for i in range(N_CORES):
        bs = slice(BPC * i, BPC * (i + 1))
        in_maps.append({
            "qT": qT[bs], "kT": kT[bs], "vT": vT[bs], "veff": veff[bs],
            "wq": wq, "wk": wk, "wv": wv, "wfc": wfc,
            "bng": bng, "bnb": bnb,
        })
    res = run_bass_kernel_spmd(nc, in_maps, core_ids=list(range(N_CORES)))
    global LAST_RESULTS
    LAST_RESULTS = res
    out = np.concatenate([res.results[i]["out"] for i in range(N_CORES)], axis=0)
    return out.reshape(B, C, HH, WW).astype(np.float32)



# revision 32
# speedup vs baseline: 1.1124x; 1.1124x over previous
"""Trainium2 Bass kernel for nn_MultiHeadAttention (channel-attention transformer block).

Math (per batch b, with X* = reshape(*, [C, P]), P = 4096, C = 128, D = 512):
  Q = Xq @ (Wq/temp)^T, K = Xk @ Wk^T, V = Xv @ Wv^T            [C, D]
  per head h (8 heads, ld=64): A_h = softmax(Q_h K_h^T); O_h = A_h V_h
  O = silu(O); O = (O - mean)/(unbiased_std + eps)   (LN affine folded into fc)
  out_pre = (v + Wfc@ln_beta) + O @ (Wfc*ln_gamma)^T
  out = BatchNorm2d(out_pre)   (batch stats over (b,h,w), biased var)

Sharding: data-parallel over batch, 2 batches per core on 8 cores; BatchNorm
statistics combined with a tiny AllReduce ([128,2] per core).

v2 design:
  - weights quantized to fp8 E3M4 with power-of-2 scales folded into existing
    per-row scalars (exp scale, sigmoid scale, LN-sqrt scale) => zero extra ops
  - activations/outputs in bf16 (DMA ~19MB/core vs 50MB f32)
  - Q/K projections computed weight-stationary so QT/KT land pre-transposed
  - attention ops batched across all 8 heads (1 exp, 1 reduce, 1 aT copy)
  - BN partial sums via accum_out on the residual-add + Square passes
  - bf16 output, upcast on host
"""

import os

import numpy as np
import ml_dtypes

import concourse.mybir as mybir
import concourse.tile as tile
from concourse import bacc
from concourse.bass_utils import run_bass_kernel_spmd
from concourse.masks import make_identity

# ---- problem constants (hardcoded per contract) ----
B, C, HH, WW = 16, 128, 64, 64
P = HH * WW           # 4096
NH, LD = 8, 64
D = NH * LD           # 512
N_CORES = 8
BPC = B // N_CORES    # 2 batches per core
NCH = P // 128        # 32 pixel chunks (contraction)
NPT = P // 512        # 8 output column tiles for fc
LN_EPS = 1e-6
BN_EPS = 1e-5
F32 = mybir.dt.float32
BF16 = mybir.dt.bfloat16
W8MODE = os.environ.get("BASS_W8", "e3")  # e3 | e4 | bf16
FP8 = {"e3": mybir.dt.float8e3, "e4": mybir.dt.float8e4,
       "bf16": mybir.dt.bfloat16}[W8MODE]
W8BYTES = 2 if W8MODE == "bf16" else 1
FP8_MAX_TARGET = {"e3": 14.0, "e4": 224.0, "bf16": 14.0}[W8MODE]

_BUILD_CACHE: dict = {}
LAST_RESULTS = None  # BassKernelResults of the most recent run (for profiling)

# host-side fp8 scales (power of two), computed at pack time, baked into build
_SCALES: dict = {}


def _emit(ctx, nc, tc, io, scales):
    PH = int(os.environ.get("BASS_PHASES", "9"))
    AF = mybir.ActivationFunctionType
    ALU = mybir.AluOpType
    AX = mybir.AxisListType
    s_q, s_k, s_v, s_fc = (scales[k] for k in ("s_q", "s_k", "s_v", "s_fc"))

    consts = ctx.enter_context(tc.tile_pool(name="consts", bufs=1))
    big = ctx.enter_context(tc.tile_pool(name="big", bufs=1))
    sb = ctx.enter_context(tc.tile_pool(name="sb", bufs=2))
    small = ctx.enter_context(tc.tile_pool(name="small", bufs=4))
    stat = ctx.enter_context(tc.tile_pool(name="stat", bufs=1))
    dram = ctx.enter_context(tc.tile_pool(name="dram", bufs=1, space="DRAM"))

    # identity for PE transposes (f32 master, bf16 working copy); a dummy
    # transpose primes PE's view of the identity writer so later transposes
    # carry a single sync wait (HW allows 1 per instruction)
    ident_f = consts.tile([128, 128], F32, tag="identf", name="identf")
    make_identity(nc, ident_f)
    ident = consts.tile([128, 128], BF16, tag="ident", name="ident")
    nc.vector.tensor_copy(out=ident, in_=ident_f)

    bng = consts.tile([128, 1], F32, tag="bng", name="bng")
    bnb = consts.tile([128, 1], F32, tag="bnb", name="bnb")
    epsbn = consts.tile([128, 1], F32, tag="epsbn", name="epsbn")
    nc.gpsimd.dma_start(out=bng, in_=io["bng"][:, :])
    nc.gpsimd.dma_start(out=bnb, in_=io["bnb"][:, :])
    nc.vector.memset(epsbn, BN_EPS)

    # ---- prefetch: everything lives in SBUF (quartered DMAs so compute can
    # start as soon as the first chunks land)
    qa_sb = big.tile([128, NCH, 2, 128], BF16, tag="qa_sb", name="qa_sb")
    ka_sb = big.tile([128, NCH, 2, 128], BF16, tag="ka_sb", name="ka_sb")
    va_sb = big.tile([128, NCH, 2, 128], BF16, tag="va_sb", name="va_sb")
    wq_sb = big.tile([128, NCH, 4, 128], FP8, tag="wq_sb", name="wq_sb")
    wk_sb = big.tile([128, NCH, 4, 128], FP8, tag="wk_sb", name="wk_sb")
    wv_sb = big.tile([128, NCH, 512], FP8, tag="wv_sb", name="wv_sb")
    QTR = NCH // 4
    for q4 in range(4):
        cs = slice(QTR * q4, QTR * (q4 + 1))
        nc.sync.dma_start(out=qa_sb[:, cs, :, :], in_=io["qa"][:, cs, :, :])
        nc.scalar.dma_start(out=wq_sb[:, cs, :, :], in_=io["wq"][:, cs, :, :])
    for q4 in range(4):
        cs = slice(QTR * q4, QTR * (q4 + 1))
        nc.sync.dma_start(out=ka_sb[:, cs, :, :], in_=io["ka"][:, cs, :, :])
        nc.scalar.dma_start(out=wk_sb[:, cs, :, :], in_=io["wk"][:, cs, :, :])
    for q4 in range(4):
        cs = slice(QTR * q4, QTR * (q4 + 1))
        nc.sync.dma_start(out=va_sb[:, cs, :, :], in_=io["va"][:, cs, :, :])
        nc.scalar.dma_start(out=wv_sb[:, cs, :], in_=io["wv"][:, cs, :])

    # residual (+ folded fc bias) and fc weights on the gpsimd queue
    veff_sb = []
    for b in range(BPC):
        t = big.tile([128, P], BF16, tag=f"veff{b}", name=f"veff{b}")
        nc.gpsimd.dma_start(out=t, in_=io["veff"][b, :, :])
        veff_sb.append(t)
    wfc_sb = big.tile([128, NPT, 4, 512], FP8, tag="wfc_sb", name="wfc_sb")
    nc.gpsimd.dma_start(out=wfc_sb, in_=io["wfc"])
    out_sb = [big.tile([128, P], BF16, tag=f"outb{b}", name=f"outb{b}")
              for b in range(BPC)]

    # ---- phase A: QKV projections, accumulating over the P=4096 contraction.
    # Q,K are weight-stationary (outputs arrive transposed: [d, c]); V is
    # activation-stationary (output [c, d], the layout attention needs).
    # Each open accumulation group owns a full PSUM bank; d-chunks go
    # sequentially with pool rotation.
    ctx_a1 = tc.tile_pool(name="ps_qk", bufs=2, space="PSUM")
    ps_qk = ctx_a1.__enter__()
    ctx_a2 = tc.tile_pool(name="ps_v", bufs=1, space="PSUM")
    ps_v = ctx_a2.__enter__()
    warm = ps_v.tile([128, 128], BF16, tag="warm", name="warm")
    nc.tensor.transpose(warm[:, :], ident[:, :], ident[:, :])

    qkv_sb = []
    for b in range(BPC):
        QT_sb = sb.tile([128, 512], BF16, tag=f"QT_sb{b}", name=f"QT_sb{b}")
        KT_sb = sb.tile([128, 512], BF16, tag=f"KT_sb{b}", name=f"KT_sb{b}")
        V_sb = sb.tile([128, 512], BF16, tag=f"V_sb{b}", name=f"V_sb{b}")
        qkv_sb.append((QT_sb, KT_sb, V_sb))

    if PH < 2:
        for b in range(BPC):
            nc.vector.memset(out_sb[b], 0.0)
            eng = nc.sync if b == 0 else nc.scalar
            eng.dma_start(out=io["out"][b, :, :], in_=out_sb[b][:, :])
        return
    for w_sb, a_sb, which in ((wq_sb, qa_sb, 0), (wk_sb, ka_sb, 1)):
        for dc in range(4):
            ps = [ps_qk.tile([128, 128], F32, tag=f"qk{b}", name=f"qk{b}")
                  for b in range(BPC)]
            for chunk in range(NCH):
                for b in range(BPC):
                    nc.tensor.matmul(ps[b][:, :], w_sb[:, chunk, dc, :],
                                     a_sb[:, chunk, b, :],
                                     start=chunk == 0, stop=chunk == NCH - 1)
            for b in range(BPC):
                dst = qkv_sb[b][which]
                if b == 0:
                    nc.vector.tensor_copy(out=dst[:, dc * 128:(dc + 1) * 128],
                                          in_=ps[b][:, :])
                else:
                    nc.scalar.copy(out=dst[:, dc * 128:(dc + 1) * 128],
                                   in_=ps[b][:, :])
    for b in range(BPC):
        vp = ps_v.tile([128, 512], F32, tag=f"vp{b}", name=f"vp{b}")
        for chunk in range(NCH):
            nc.tensor.matmul(vp[:, :], va_sb[:, chunk, b, :], wv_sb[:, chunk, :],
                             start=chunk == 0, stop=chunk == NCH - 1)
        nc.vector.tensor_copy(out=qkv_sb[b][2], in_=vp[:, :])
    ctx_a2.__exit__(None, None, None)
    ctx_a1.__exit__(None, None, None)

    ps_s = ctx.enter_context(tc.tile_pool(name="ps_s", bufs=1, space="PSUM"))
    ps_o = ctx.enter_context(tc.tile_pool(name="ps_o", bufs=2, space="PSUM"))
    ps_fc = ctx.enter_context(tc.tile_pool(name="ps_fc", bufs=2, space="PSUM"))

    # per-channel partial sums: cols 0..15 = sum(out) per (b,pt), 16..31 = sum(out^2)
    pcols = stat.tile([128, 32], F32, tag="pcols", name="pcols")

    exp_scale = 1.0 / (s_q * s_k)
    sig_scale = 1.0 / s_v
    sqrt_scale = (float(D) / (D - 1)) * s_fc * s_fc
    eps_s = LN_EPS * s_v * s_fc

    dbg = os.environ.get("BASS_DEBUG_DUMP", "0") == "1" and "dbg_qt" in io
    if dbg:
        for b in range(BPC):
            nc.gpsimd.dma_start(out=io["dbg_qt"][b], in_=qkv_sb[b][0][:, :, :])
            nc.gpsimd.dma_start(out=io["dbg_kt"][b], in_=qkv_sb[b][1][:, :, :])
            nc.gpsimd.dma_start(out=io["dbg_v"][b], in_=qkv_sb[b][2][:, :])

    if PH < 3:
        for b in range(BPC):
            nc.vector.memset(out_sb[b], 0.0)
            eng = nc.sync if b == 0 else nc.scalar
            eng.dma_start(out=io["out"][b, :, :], in_=out_sb[b][:, :])
        return
    SUB = int(os.environ.get("BASS_SUB3", "99"))
    # ---- phase B: attention + silu + LN per batch (baseline-proven per-head
    # structure; scales folded into exp/sigmoid/sqrt)
    xTs = []
    for b in range(BPC):
        QT_sb, KT_sb, V_sb = qkv_sb[b]
        Opsum = ps_o.tile([128, 512], F32, tag="O", name="O")
        Osc = sb.tile([128, 512], F32, tag="Osc", name="Osc")
        for h in range(NH):
            po = (h % 2) * 64
            fo = (h // 2) * 128
            S = ps_s.tile([128, 128], F32, tag="S", name="S")
            nc.tensor.matmul(S[:, :], QT_sb[po:po + 64, fo:fo + 128],
                             KT_sb[po:po + 64, fo:fo + 128], start=True, stop=True)
            e_f = sb.tile([128, 128], F32, tag="e_f", name="e_f")
            lsum = small.tile([128, 1], F32, tag="lsum", name="lsum")
            nc.scalar.activation(out=e_f, in_=S[:, :], func=AF.Exp,
                                 scale=exp_scale, accum_out=lsum)
            rs = small.tile([128, 1], F32, tag="rs", name="rs")
            nc.vector.reciprocal(rs, lsum)
            tpa = ps_s.tile([128, 128], F32, tag="stp", name="stp")
            nc.tensor.transpose(tpa[:, :], e_f[:, :], ident_f[:, :])
            aT = sb.tile([128, 128], BF16, tag="aT", name="aT")
            nc.scalar.copy(out=aT, in_=tpa[:, :])
            nc.tensor.matmul(Opsum[:, h * 64:(h + 1) * 64], aT[:, :],
                             V_sb[:, h * 64:(h + 1) * 64], start=True, stop=True)
            nc.vector.tensor_scalar_mul(out=Osc[:, h * 64:(h + 1) * 64],
                                        in0=Opsum[:, h * 64:(h + 1) * 64],
                                        scalar1=rs)

        if PH < 4:
            xTs.append(None)
            continue
        # silu (Osc is s_v-scaled; sigmoid descales its argument) + layernorm
        sg = sb.tile([128, D], F32, tag="sg", name="sg")
        nc.scalar.activation(out=sg, in_=Osc, func=AF.Sigmoid, scale=sig_scale)
        Osw = sb.tile([128, D], F32, tag="Osw", name="Osw")
        nc.vector.tensor_mul(out=Osw, in0=Osc, in1=sg)
        st6 = small.tile([128, 6], F32, tag="st6", name="st6")
        nc.vector.bn_stats(out=st6, in_=Osw)
        mv = small.tile([128, 2], F32, tag="mv", name="mv")
        nc.vector.bn_aggr(out=mv, in_=st6)
        # sd = s_v*s_fc*(unbiased std); then += s_v*s_fc*eps; the 1/s_fc
        # factor folds fc's fp8 weight scale into xhat
        sd = small.tile([128, 1], F32, tag="sd", name="sd")
        nc.scalar.activation(out=sd, in_=mv[:, 1:2], func=AF.Sqrt, scale=sqrt_scale)
        nc.vector.tensor_scalar_add(out=sd, in0=sd, scalar1=eps_s)
        rstd = small.tile([128, 1], F32, tag="rstd", name="rstd")
        nc.vector.reciprocal(rstd, sd)
        xhat = sb.tile([128, D], BF16, tag="xhat", name="xhat")
        nc.vector.tensor_scalar(out=xhat, in0=Osw, scalar1=mv[:, 0:1], scalar2=rstd,
                                op0=ALU.subtract, op1=ALU.mult)
        xT = sb.tile([128, 4, 128], BF16, tag="xT", name="xT")
        for dc in range(4):
            tp = ps_s.tile([128, 128], BF16, tag="xtp", name="xtp")
            nc.tensor.transpose(tp[:, :], xhat[:, dc * 128:(dc + 1) * 128],
                                ident[:, :])
            nc.vector.tensor_copy(out=xT[:, dc, :], in_=tp[:, :])
        xTs.append(xT)
        if dbg:
            nc.gpsimd.dma_start(out=io["dbg_osc"][b], in_=Osc[:, :])
            nc.gpsimd.dma_start(out=io["dbg_xhat"][b], in_=xhat[:, :])

    if PH < 5:
        for b in range(BPC):
            nc.vector.memset(out_sb[b], 0.0)
            eng = nc.sync if b == 0 else nc.scalar
            eng.dma_start(out=io["out"][b, :, :], in_=out_sb[b][:, :])
        return
    # ---- phase D: fc + residual + BN partial sums
    junk = sb.tile([128, 512], BF16, tag="junk", name="junk")
    for b in range(BPC):
        for pt in range(NPT):
            O2 = ps_fc.tile([128, 512], F32, tag="O2", name="O2")
            for dc in range(4):
                nc.tensor.matmul(O2[:, :], xTs[b][:, dc, :], wfc_sb[:, pt, dc, :],
                                 start=dc == 0, stop=dc == 3)
            seg = out_sb[b][:, pt * 512:(pt + 1) * 512]
            i = b * NPT + pt
            if PH >= 6:
                nc.vector.scalar_tensor_tensor(
                    out=seg, in0=O2[:, :], scalar=1.0,
                    in1=veff_sb[b][:, pt * 512:(pt + 1) * 512],
                    op0=ALU.mult, op1=ALU.add, accum_out=pcols[:, i:i + 1])
            else:
                nc.vector.tensor_add(out=seg, in0=O2[:, :],
                                     in1=veff_sb[b][:, pt * 512:(pt + 1) * 512])
                nc.vector.reduce_sum(pcols[:, i:i + 1], seg, axis=AX.X)
            nc.scalar.activation(out=junk, in_=seg, func=AF.Square,
                                 accum_out=pcols[:, 16 + i:17 + i])

    # ---- phase E: BN stats AllReduce + normalize + store
    stats2 = stat.tile([128, 2], F32, tag="stats2", name="stats2")
    nc.vector.reduce_sum(stats2[:, 0:1], pcols[:, 0:16], axis=AX.X)
    nc.vector.reduce_sum(stats2[:, 1:2], pcols[:, 16:32], axis=AX.X)

    cin = dram.tile([128, 2], F32, tag="cin", name="cin")
    cout = dram.tile([128, 2], F32, tag="cout", name="cout")
    nc.gpsimd.dma_start(out=cin[:, :], in_=stats2)
    if os.environ.get("BASS_SKIP_COLL", "0") == "1":
        nc.gpsimd.dma_start(out=cout[:, :], in_=cin[:, :])
    else:
        nc.gpsimd.collective_compute(
            "AllReduce",
            mybir.AluOpType.add,
            replica_groups=[list(range(N_CORES))],
            ins=[cin.opt()],
            outs=[cout.opt()],
        )
    red = stat.tile([128, 2], F32, tag="red", name="red")
    nc.gpsimd.dma_start(out=red[:, :], in_=cout[:, :])

    inv_n = 1.0 / float(B * P)
    mean = small.tile([128, 1], F32, tag="mean", name="mean")
    nc.scalar.mul(out=mean, in_=red[:, 0:1], mul=inv_n)
    ex2 = small.tile([128, 1], F32, tag="ex2", name="ex2")
    nc.vector.tensor_scalar_mul(out=ex2, in0=red[:, 1:2], scalar1=inv_n)
    msq = small.tile([128, 1], F32, tag="msq", name="msq")
    nc.vector.tensor_mul(out=msq, in0=mean, in1=mean)
    var = small.tile([128, 1], F32, tag="var", name="var")
    nc.vector.tensor_sub(out=var, in0=ex2, in1=msq)
    sdv = small.tile([128, 1], F32, tag="sdv", name="sdv")
    nc.scalar.activation(out=sdv, in_=var, func=AF.Sqrt, bias=epsbn)
    invs = small.tile([128, 1], F32, tag="invs", name="invs")
    nc.vector.reciprocal(invs, sdv)
    scl = small.tile([128, 1], F32, tag="scl", name="scl")
    nc.vector.tensor_mul(out=scl, in0=bng, in1=invs)
    tmp = small.tile([128, 1], F32, tag="tmp", name="tmp")
    nc.vector.tensor_mul(out=tmp, in0=mean, in1=scl)
    shf = small.tile([128, 1], F32, tag="shf", name="shf")
    nc.vector.tensor_sub(out=shf, in0=bnb, in1=tmp)

    for b in range(BPC):
        nc.vector.tensor_scalar(out=out_sb[b][:, :], in0=out_sb[b][:, :],
                                scalar1=scl, scalar2=shf,
                                op0=ALU.mult, op1=ALU.add)
        eng = nc.sync if b == 0 else nc.scalar
        eng.dma_start(out=io["out"][b, :, :], in_=out_sb[b][:, :])


def _build(scales):
    key = (os.environ.get("BASS_SKIP_COLL", "0"), W8MODE,
           os.environ.get("BASS_PHASES", "9"), os.environ.get("BASS_SUB3", "99"),
           os.environ.get("BASS_DEBUG_DUMP", "0"), tuple(sorted(scales.items())))
    if key in _BUILD_CACHE:
        return _BUILD_CACHE[key]
    nc = bacc.Bacc("TRN2", target_bir_lowering=False, debug=False, num_devices=N_CORES)
    io = {
        "qa": nc.dram_tensor("qa", [128, NCH, 2, 128], BF16, kind="ExternalInput").ap(),
        "ka": nc.dram_tensor("ka", [128, NCH, 2, 128], BF16, kind="ExternalInput").ap(),
        "va": nc.dram_tensor("va", [128, NCH, 2, 128], BF16, kind="ExternalInput").ap(),
        "veff": nc.dram_tensor("veff", [BPC, C, P], BF16, kind="ExternalInput").ap(),
        # fp8 payloads travel as uint8 through the PJRT boundary; bitcast on
        # the device side
        "wq": _wtensor(nc, "wq", [128, NCH, 4, 128]),
        "wk": _wtensor(nc, "wk", [128, NCH, 4, 128]),
        "wv": _wtensor(nc, "wv", [128, NCH, 512]),
        "wfc": _wtensor(nc, "wfc", [128, NPT, 4, 512]),
        "bng": nc.dram_tensor("bng", [C, 1], F32, kind="ExternalInput").ap(),
        "bnb": nc.dram_tensor("bnb", [C, 1], F32, kind="ExternalInput").ap(),
        "out": nc.dram_tensor("out", [BPC, C, P], BF16, kind="ExternalOutput").ap(),
    }
    if os.environ.get("BASS_DEBUG_DUMP", "0") == "1":
        io.update({
            "dbg_qt": nc.dram_tensor("dbg_qt", [BPC, 128, 4, 128], BF16, kind="ExternalOutput").ap(),
            "dbg_kt": nc.dram_tensor("dbg_kt", [BPC, 128, 4, 128], BF16, kind="ExternalOutput").ap(),
            "dbg_v": nc.dram_tensor("dbg_v", [BPC, 128, 512], BF16, kind="ExternalOutput").ap(),
            "dbg_ef": nc.dram_tensor("dbg_ef", [BPC, 128, 8, 128], BF16, kind="ExternalOutput").ap(),
            "dbg_osc": nc.dram_tensor("dbg_osc", [BPC, 128, 512], F32, kind="ExternalOutput").ap(),
            "dbg_xhat": nc.dram_tensor("dbg_xhat", [BPC, 128, 512], BF16, kind="ExternalOutput").ap(),
        })
    from contextlib import ExitStack
    with tile.TileContext(nc) as tc, ExitStack() as ctx:
        _emit(ctx, nc, tc, io, scales)
    nc.compile()
    _BUILD_CACHE[key] = nc
    return nc


def _pow2_scale(w):
    m = float(np.abs(w).max())
    return float(2.0 ** np.floor(np.log2(FP8_MAX_TARGET / m)))


def _wtensor(nc, name, shape):
    if W8MODE == "bf16":
        return nc.dram_tensor(name, shape, mybir.dt.bfloat16,
                              kind="ExternalInput").ap()
    return nc.dram_tensor(name, shape, mybir.dt.uint8,
                          kind="ExternalInput").bitcast(FP8).ap()


def _q8(w, s):
    w = np.asarray(w, np.float32) * s
    if W8MODE == "bf16":
        return np.ascontiguousarray(w.astype(ml_dtypes.bfloat16))
    dt8 = ml_dtypes.float8_e3m4 if W8MODE == "e3" else ml_dtypes.float8_e4m3
    return np.ascontiguousarray(w.astype(dt8)).view(np.uint8)


def _bf16(x):
    return np.ascontiguousarray(np.asarray(x, np.float32).astype(ml_dtypes.bfloat16))


def _pack_acts(xf):
    # [b, c, p] f32 -> [128, NCH, b, c] bf16  (pixel-in-chunk, chunk, batch, channel)
    b = xf.shape[0]
    return _bf16(xf.transpose(2, 0, 1).reshape(NCH, 128, b, C).transpose(1, 0, 2, 3))


def kernel(v, k, q, w_qs, w_ks, w_vs, w_fc, ln_gamma, ln_beta, temperature,
           bn_gamma, bn_beta, **_ignored):
    v = np.asarray(v, np.float32)
    k = np.asarray(k, np.float32)
    q = np.asarray(q, np.float32)
    w_qs = np.asarray(w_qs, np.float32)
    w_ks = np.asarray(w_ks, np.float32)
    w_vs = np.asarray(w_vs, np.float32)
    w_fc = np.asarray(w_fc, np.float32)
    ln_gamma = np.asarray(ln_gamma, np.float32)
    ln_beta = np.asarray(ln_beta, np.float32)
    temp = float(np.asarray(temperature))
    bn_gamma = np.asarray(bn_gamma, np.float32)
    bn_beta = np.asarray(bn_beta, np.float32)

    qf = q.reshape(B, C, P)
    kf = k.reshape(B, C, P)
    vf = v.reshape(B, C, P)
    qa = _pack_acts(qf)   # [32, 128, 16, 128]
    ka = _pack_acts(kf)
    va = _pack_acts(vf)

    wqT = (w_qs / temp).T            # [P, D]
    wkT = w_ks.T
    wvT = w_vs.T
    wfcT_eff = (w_fc * ln_gamma[None, :]).T   # [D, P]
    s_q = _pow2_scale(wqT)
    s_k = _pow2_scale(wkT)
    s_v = _pow2_scale(wvT)
    s_fc = _pow2_scale(wfcT_eff)
    scales = {"s_q": s_q, "s_k": s_k, "s_v": s_v, "s_fc": s_fc}

    wq = _q8(wqT.reshape(NCH, 128, 4, 128).transpose(1, 0, 2, 3), s_q)
    wk = _q8(wkT.reshape(NCH, 128, 4, 128).transpose(1, 0, 2, 3), s_k)
    wv = _q8(wvT.reshape(NCH, 128, 512).transpose(1, 0, 2), s_v)
    wfc = _q8(wfcT_eff.reshape(4, 128, NPT, 512).transpose(1, 2, 0, 3), s_fc)
    bias_fc = (w_fc @ ln_beta).astype(np.float32)
    veff = vf + bias_fc[None, None, :]
    bng = np.ascontiguousarray(bn_gamma.reshape(C, 1))
    bnb = np.ascontiguousarray(bn_beta.reshape(C, 1))

    nc = _build(scales)
    in_maps = []
    for i in range(N_CORES):
        bs = slice(BPC * i, BPC * (i + 1))
        in_maps.append({
            "qa": np.ascontiguousarray(qa[:, :, bs, :]),
            "ka": np.ascontiguousarray(ka[:, :, bs, :]),
            "va": np.ascontiguousarray(va[:, :, bs, :]),
            # dims: [128 pixel-in-chunk, 32 chunk, 2 batch, 128 channel]
            "veff": _bf16(veff[bs]),
            "wq": wq, "wk": wk, "wv": wv, "wfc": wfc,
            "bng": bng, "bnb": bnb,
        })
    res = run_bass_kernel_spmd(nc, in_maps, core_ids=list(range(N_CORES)))
    global LAST_RESULTS
    LAST_RESULTS = res
    out = np.concatenate([np.asarray(res.results[i]["out"], dtype=np.float32)
                          for i in range(N_CORES)], axis=0)
    return out.reshape(B, C, HH, WW)


MODE = f"v2-{W8MODE}w-bf16a"


# revision 36
# speedup vs baseline: 1.2366x; 1.1116x over previous
"""Trainium2 Bass kernel for nn_MultiHeadAttention (channel-attention transformer block).

Math (per batch b, with X* = reshape(*, [C, P]), P = 4096, C = 128, D = 512):
  Q = Xq @ (Wq/temp)^T, K = Xk @ Wk^T, V = Xv @ Wv^T            [C, D]
  per head h (8 heads, ld=64): A_h = softmax(Q_h K_h^T); O_h = A_h V_h
  O = silu(O); O = (O - mean)/(unbiased_std + eps)   (LN affine folded into fc)
  out_pre = (v + Wfc@ln_beta) + O @ (Wfc*ln_gamma)^T
  out = BatchNorm2d(out_pre)   (batch stats over (b,h,w), biased var)

Sharding: data-parallel over batch, 2 batches per core on 8 cores; BatchNorm
statistics combined with a tiny AllReduce ([128,2] per core).

v2 design:
  - weights quantized to fp8 E3M4 with power-of-2 scales folded into existing
    per-row scalars (exp scale, sigmoid scale, LN-sqrt scale) => zero extra ops
  - activations/outputs in bf16 (DMA ~19MB/core vs 50MB f32)
  - Q/K projections computed weight-stationary so QT/KT land pre-transposed
  - attention ops batched across all 8 heads (1 exp, 1 reduce, 1 aT copy)
  - BN partial sums via accum_out on the residual-add + Square passes
  - bf16 output, upcast on host
"""

import os

import numpy as np
import ml_dtypes

import concourse.mybir as mybir
import concourse.tile as tile
from concourse import bacc
from concourse.bass_utils import run_bass_kernel_spmd
from concourse.masks import make_identity

# ---- problem constants (hardcoded per contract) ----
B, C, HH, WW = 16, 128, 64, 64
P = HH * WW           # 4096
NH, LD = 8, 64
D = NH * LD           # 512
N_CORES = 8
BPC = B // N_CORES    # 2 batches per core
NCH = P // 128        # 32 pixel chunks (contraction)
NPT = P // 512        # 8 output column tiles for fc
LN_EPS = 1e-6
BN_EPS = 1e-5
F32 = mybir.dt.float32
BF16 = mybir.dt.bfloat16
W8MODE = os.environ.get("BASS_W8", "e3")  # e3 | e4 | bf16
FP8 = {"e3": mybir.dt.float8e3, "e4": mybir.dt.float8e4,
       "bf16": mybir.dt.bfloat16}[W8MODE]
W8BYTES = 2 if W8MODE == "bf16" else 1
FP8_MAX_TARGET = {"e3": 14.0, "e4": 224.0, "bf16": 14.0}[W8MODE]

_BUILD_CACHE: dict = {}
LAST_RESULTS = None  # BassKernelResults of the most recent run (for profiling)

# host-side fp8 scales (power of two), computed at pack time, baked into build
_SCALES: dict = {}


def _emit(ctx, nc, tc, io, scales):
    PH = int(os.environ.get("BASS_PHASES", "9"))
    AF = mybir.ActivationFunctionType
    ALU = mybir.AluOpType
    AX = mybir.AxisListType
    s_q, s_k, s_v, s_fc = (scales[k] for k in ("s_q", "s_k", "s_v", "s_fc"))

    consts = ctx.enter_context(tc.tile_pool(name="consts", bufs=1))
    big = ctx.enter_context(tc.tile_pool(name="big", bufs=1))
    sb = ctx.enter_context(tc.tile_pool(name="sb", bufs=2))
    small = ctx.enter_context(tc.tile_pool(name="small", bufs=4))
    stat = ctx.enter_context(tc.tile_pool(name="stat", bufs=1))
    dram = ctx.enter_context(tc.tile_pool(name="dram", bufs=1, space="DRAM"))

    ident = consts.tile([128, 128], BF16, tag="ident", name="ident")
    identf = consts.tile([128, 128], F32, tag="identf", name="identf")
    make_identity(nc, identf)
    nc.vector.tensor_copy(out=ident, in_=identf)
    ones = consts.tile([128, 1], BF16, tag="ones", name="ones")
    nc.vector.memset(ones, 1.0)

    bng = consts.tile([128, 1], F32, tag="bng", name="bng")
    bnb = consts.tile([128, 1], F32, tag="bnb", name="bnb")
    epsbn = consts.tile([128, 1], F32, tag="epsbn", name="epsbn")
    nc.gpsimd.dma_start(out=bng, in_=io["bng"][:, :])
    nc.gpsimd.dma_start(out=bnb, in_=io["bnb"][:, :])
    nc.vector.memset(epsbn, BN_EPS)

    # ---- prefetch: everything lives in SBUF (quartered DMAs so compute can
    # start as soon as the first chunks land)
    qa_sb = big.tile([128, NCH, 2, 128], BF16, tag="qa_sb", name="qa_sb")
    ka_sb = big.tile([128, NCH, 2, 128], BF16, tag="ka_sb", name="ka_sb")
    va_sb = big.tile([128, NCH, 2, 128], BF16, tag="va_sb", name="va_sb")
    wq_sb = big.tile([128, NCH, 4, 128], FP8, tag="wq_sb", name="wq_sb")
    wk_sb = big.tile([128, NCH, 4, 128], FP8, tag="wk_sb", name="wk_sb")
    wv_sb = big.tile([128, NCH, 512], FP8, tag="wv_sb", name="wv_sb")
    QTR = NCH // 4
    for q4 in range(4):
        cs = slice(QTR * q4, QTR * (q4 + 1))
        nc.sync.dma_start(out=qa_sb[:, cs, :, :], in_=io["qa"][:, cs, :, :])
        nc.scalar.dma_start(out=wq_sb[:, cs, :, :], in_=io["wq"][:, cs, :, :])
    for q4 in range(4):
        cs = slice(QTR * q4, QTR * (q4 + 1))
        nc.sync.dma_start(out=ka_sb[:, cs, :, :], in_=io["ka"][:, cs, :, :])
        nc.scalar.dma_start(out=wk_sb[:, cs, :, :], in_=io["wk"][:, cs, :, :])
    for q4 in range(4):
        cs = slice(QTR * q4, QTR * (q4 + 1))
        nc.sync.dma_start(out=va_sb[:, cs, :, :], in_=io["va"][:, cs, :, :])
        nc.scalar.dma_start(out=wv_sb[:, cs, :], in_=io["wv"][:, cs, :])

    # residual (+ folded fc bias) and fc weights on the gpsimd queue
    veff_sb = []
    for b in range(BPC):
        t = big.tile([128, P], BF16, tag=f"veff{b}", name=f"veff{b}")
        nc.gpsimd.dma_start(out=t, in_=io["veff"][b, :, :])
        veff_sb.append(t)
    wfc_sb = big.tile([128, NPT, 4, 512], FP8, tag="wfc_sb", name="wfc_sb")
    nc.gpsimd.dma_start(out=wfc_sb, in_=io["wfc"])
    out_sb = [big.tile([128, P], BF16, tag=f"outb{b}", name=f"outb{b}")
              for b in range(BPC)]

    # ---- phase A: QKV projections, accumulating over the P=4096 contraction.
    # Q,K weight-stationary with both batches as one 256-wide moving operand
    # (outputs arrive transposed [d, b|c]); V activation-stationary 512-wide.
    ctx_a1 = tc.tile_pool(name="ps_qk", bufs=2, space="PSUM")
    ps_qk = ctx_a1.__enter__()
    ctx_a2 = tc.tile_pool(name="ps_v", bufs=1, space="PSUM")
    ps_v = ctx_a2.__enter__()
    warm = ps_v.tile([128, 128], BF16, tag="warm", name="warm")
    nc.tensor.transpose(warm[:, :], ident[:, :], ident[:, :])

    qkv_sb = []
    for b in range(BPC):
        QT_sb = sb.tile([128, 512], BF16, tag=f"QT_sb{b}", name=f"QT_sb{b}")
        KT_sb = sb.tile([128, 512], BF16, tag=f"KT_sb{b}", name=f"KT_sb{b}")
        V_sb = sb.tile([128, 512], BF16, tag=f"V_sb{b}", name=f"V_sb{b}")
        qkv_sb.append((QT_sb, KT_sb, V_sb))

    if PH >= 2:
        for w_sb, a_sb, which in ((wq_sb, qa_sb, 0), (wk_sb, ka_sb, 1)):
            for dc in range(4):
                pq = ps_qk.tile([128, 256], F32, tag="qkp", name="qkp")
                for chunk in range(NCH):
                    nc.tensor.matmul(pq[:, :], w_sb[:, chunk, dc, :],
                                     a_sb[:, chunk, :, :],
                                     start=chunk == 0, stop=chunk == NCH - 1)
                fo = dc * 128
                dst0 = qkv_sb[0][which]
                dst1 = qkv_sb[1][which]
                nc.vector.tensor_copy(out=dst0[:, fo:fo + 128], in_=pq[:, 0:128])
                nc.scalar.copy(out=dst1[:, fo:fo + 128], in_=pq[:, 128:256])
        for b in range(BPC):
            vp = ps_v.tile([128, 512], F32, tag=f"vp{b}", name=f"vp{b}")
            for chunk in range(NCH):
                nc.tensor.matmul(vp[:, :], va_sb[:, chunk, b, :],
                                 wv_sb[:, chunk, :],
                                 start=chunk == 0, stop=chunk == NCH - 1)
            if b == 0:
                nc.vector.tensor_copy(out=qkv_sb[b][2], in_=vp[:, :])
            else:
                nc.scalar.copy(out=qkv_sb[b][2], in_=vp[:, :])
    ctx_a2.__exit__(None, None, None)
    ctx_a1.__exit__(None, None, None)

    ps_s = ctx.enter_context(tc.tile_pool(name="ps_s", bufs=2, space="PSUM"))
    ps_xt = ctx.enter_context(tc.tile_pool(name="ps_xt", bufs=1, space="PSUM"))
    ps_o = ctx.enter_context(tc.tile_pool(name="ps_o", bufs=1, space="PSUM"))
    ps_fc = ctx.enter_context(tc.tile_pool(name="ps_fc", bufs=2, space="PSUM"))

    # per-channel partial sums: cols 0..15 = sum(out) per (b,pt), 16..31 = sum(out^2)
    pcols = stat.tile([128, 32], F32, tag="pcols", name="pcols")

    exp_scale = 1.0 / (s_q * s_k)
    sig_scale = 1.0 / s_v
    sqrt_scale = (float(D) / (D - 1)) * s_fc * s_fc
    eps_s = LN_EPS * s_v * s_fc

    dbg = os.environ.get("BASS_DEBUG_DUMP", "0") == "1" and "dbg_qt" in io
    if dbg:
        for b in range(BPC):
            nc.gpsimd.dma_start(out=io["dbg_qt"][b], in_=qkv_sb[b][0][:, :])
            nc.gpsimd.dma_start(out=io["dbg_kt"][b], in_=qkv_sb[b][1][:, :])
            nc.gpsimd.dma_start(out=io["dbg_v"][b], in_=qkv_sb[b][2][:, :])

    if PH < 3:
        for b in range(BPC):
            nc.vector.memset(out_sb[b], 0.0)
            eng = nc.sync if b == 0 else nc.scalar
            eng.dma_start(out=io["out"][b, :, :], in_=out_sb[b][:, :])
        return

    # ---- phase B: attention, ST formulation. ST_h = K_h Q_h^T comes out
    # [e, c]; exp(ST) is A^T which is exactly the AV stationary operand, so no
    # PE transposes or PSUM->SBUF shuffles are needed. Softmax denominators
    # come from a 1-wide matmul against a ones vector (same stationary).
    # All exps are contiguous so the ACT Exp table loads once.
    ls = ps_o.tile([128, 16], F32, tag="ls", name="ls")
    Ops = []
    Oscs = []
    for b in range(BPC):
        QT_sb, KT_sb, V_sb = qkv_sb[b]
        Opsum = ps_o.tile([128, 512], F32, tag=f"O{b}", name=f"O{b}")
        Ops.append(Opsum)
        efts = []
        for h in range(NH):
            po = (h % 2) * 64
            fo = (h // 2) * 128
            ST = ps_s.tile([128, 128], F32, tag="S", name="S")
            nc.tensor.matmul(ST[:, :], KT_sb[po:po + 64, fo:fo + 128],
                             QT_sb[po:po + 64, fo:fo + 128], start=True, stop=True)
            eft = sb.tile([128, 128], BF16, tag="eft", name="eft")
            nc.scalar.activation(out=eft, in_=ST[:, :], func=AF.Exp,
                                 scale=exp_scale)
            efts.append(eft)
        for h in range(NH):
            nc.tensor.matmul(Opsum[:, h * 64:(h + 1) * 64], efts[h][:, :],
                             V_sb[:, h * 64:(h + 1) * 64], start=True, stop=True)
            nc.tensor.matmul(ls[:, b * 8 + h:b * 8 + h + 1], efts[h][:, :],
                             ones[:, :], start=True, stop=True)

    for b in range(BPC):
        rs = small.tile([128, 8], F32, tag="rs", name="rs")
        nc.vector.reciprocal(rs, ls[:, b * 8:b * 8 + 8])
        Osc = sb.tile([128, 512], F32, tag=f"Osc{b}", name=f"Osc{b}")
        for h in range(NH):
            nc.vector.tensor_scalar_mul(out=Osc[:, h * 64:(h + 1) * 64],
                                        in0=Ops[b][:, h * 64:(h + 1) * 64],
                                        scalar1=rs[:, h:h + 1])
        Oscs.append(Osc)
        if dbg:
            nc.gpsimd.dma_start(out=io["dbg_osc"][b], in_=Osc[:, :])

    if PH < 4:
        for b in range(BPC):
            nc.vector.memset(out_sb[b], 0.0)
            eng = nc.sync if b == 0 else nc.scalar
            eng.dma_start(out=io["out"][b, :, :], in_=out_sb[b][:, :])
        return

    # ---- phase C: silu + LN (sigmoids grouped, then sqrts, for table reuse)
    Osws = []
    mvs = []
    for b in range(BPC):
        sg = sb.tile([128, D], F32, tag=f"sg{b}", name=f"sg{b}")
        nc.scalar.activation(out=sg, in_=Oscs[b], func=AF.Sigmoid, scale=sig_scale)
        Osw = sb.tile([128, D], F32, tag=f"Osw{b}", name=f"Osw{b}")
        nc.vector.tensor_mul(out=Osw, in0=Oscs[b], in1=sg)
        st6 = small.tile([128, 6], F32, tag="st6", name="st6")
        nc.vector.bn_stats(out=st6, in_=Osw)
        mv = small.tile([128, 2], F32, tag=f"mv{b}", name=f"mv{b}")
        nc.vector.bn_aggr(out=mv, in_=st6)
        Osws.append(Osw)
        mvs.append(mv)
    xTs = []
    for b in range(BPC):
        sd = small.tile([128, 1], F32, tag="sd", name="sd")
        nc.scalar.activation(out=sd, in_=mvs[b][:, 1:2], func=AF.Sqrt,
                             scale=sqrt_scale)
        nc.vector.tensor_scalar_add(out=sd, in0=sd, scalar1=eps_s)
        rstd = small.tile([128, 1], F32, tag="rstd", name="rstd")
        nc.vector.reciprocal(rstd, sd)
        xhat = sb.tile([128, D], BF16, tag=f"xhat{b}", name=f"xhat{b}")
        nc.vector.tensor_scalar(out=xhat, in0=Osws[b], scalar1=mvs[b][:, 0:1],
                                scalar2=rstd, op0=ALU.subtract, op1=ALU.mult)
        xT = sb.tile([128, 4, 128], BF16, tag=f"xT{b}", name=f"xT{b}")
        for dc in range(4):
            tp = ps_xt.tile([128, 128], BF16, tag="xtp", name="xtp")
            nc.tensor.transpose(tp[:, :], xhat[:, dc * 128:(dc + 1) * 128],
                                ident[:, :])
            if dc % 2 == 0:
                nc.vector.tensor_copy(out=xT[:, dc, :], in_=tp[:, :])
            else:
                nc.scalar.copy(out=xT[:, dc, :], in_=tp[:, :])
        xTs.append(xT)
        if dbg:
            nc.gpsimd.dma_start(out=io["dbg_xhat"][b], in_=xhat[:, :])

    if PH < 5:
        for b in range(BPC):
            nc.vector.memset(out_sb[b], 0.0)
            eng = nc.sync if b == 0 else nc.scalar
            eng.dma_start(out=io["out"][b, :, :], in_=out_sb[b][:, :])
        return

    # ---- phase D: fc + residual + BN partial sums
    junk = sb.tile([128, 512], BF16, tag="junk", name="junk")
    for b in range(BPC):
        for pt in range(NPT):
            O2 = ps_fc.tile([128, 512], F32, tag="O2", name="O2")
            for dc in range(4):
                nc.tensor.matmul(O2[:, :], xTs[b][:, dc, :], wfc_sb[:, pt, dc, :],
                                 start=dc == 0, stop=dc == 3)
            seg = out_sb[b][:, pt * 512:(pt + 1) * 512]
            i = b * NPT + pt
            nc.vector.scalar_tensor_tensor(
                out=seg, in0=O2[:, :], scalar=1.0,
                in1=veff_sb[b][:, pt * 512:(pt + 1) * 512],
                op0=ALU.mult, op1=ALU.add, accum_out=pcols[:, i:i + 1])
            nc.scalar.activation(out=junk, in_=seg, func=AF.Square,
                                 accum_out=pcols[:, 16 + i:17 + i])

    # ---- phase E: BN stats AllReduce + normalize + store
    stats2 = stat.tile([128, 2], F32, tag="stats2", name="stats2")
    nc.vector.reduce_sum(stats2[:, 0:1], pcols[:, 0:16], axis=AX.X)
    nc.vector.reduce_sum(stats2[:, 1:2], pcols[:, 16:32], axis=AX.X)

    cin = dram.tile([128, 2], F32, tag="cin", name="cin")
    cout = dram.tile([128, 2], F32, tag="cout", name="cout")
    nc.gpsimd.dma_start(out=cin[:, :], in_=stats2)
    if os.environ.get("BASS_SKIP_COLL", "0") == "1":
        nc.gpsimd.dma_start(out=cout[:, :], in_=cin[:, :])
    else:
        nc.gpsimd.collective_compute(
            "AllReduce",
            mybir.AluOpType.add,
            replica_groups=[list(range(N_CORES))],
            ins=[cin.opt()],
            outs=[cout.opt()],
        )
    red = stat.tile([128, 2], F32, tag="red", name="red")
    nc.gpsimd.dma_start(out=red[:, :], in_=cout[:, :])

    inv_n = 1.0 / float(B * P)
    mean = small.tile([128, 1], F32, tag="mean", name="mean")
    nc.scalar.mul(out=mean, in_=red[:, 0:1], mul=inv_n)
    ex2 = small.tile([128, 1], F32, tag="ex2", name="ex2")
    nc.vector.tensor_scalar_mul(out=ex2, in0=red[:, 1:2], scalar1=inv_n)
    msq = small.tile([128, 1], F32, tag="msq", name="msq")
    nc.vector.tensor_mul(out=msq, in0=mean, in1=mean)
    var = small.tile([128, 1], F32, tag="var", name="var")
    nc.vector.tensor_sub(out=var, in0=ex2, in1=msq)
    sdv = small.tile([128, 1], F32, tag="sdv", name="sdv")
    nc.scalar.activation(out=sdv, in_=var, func=AF.Sqrt, bias=epsbn)
    invs = small.tile([128, 1], F32, tag="invs", name="invs")
    nc.vector.reciprocal(invs, sdv)
    scl = small.tile([128, 1], F32, tag="scl", name="scl")
    nc.vector.tensor_mul(out=scl, in0=bng, in1=invs)
    tmp = small.tile([128, 1], F32, tag="tmp", name="tmp")
    nc.vector.tensor_mul(out=tmp, in0=mean, in1=scl)
    shf = small.tile([128, 1], F32, tag="shf", name="shf")
    nc.vector.tensor_sub(out=shf, in0=bnb, in1=tmp)

    for b in range(BPC):
        nc.vector.tensor_scalar(out=out_sb[b][:, :], in0=out_sb[b][:, :],
                                scalar1=scl, scalar2=shf,
                                op0=ALU.mult, op1=ALU.add)
        eng = nc.sync if b == 0 else nc.scalar
        eng.dma_start(out=io["out"][b, :, :], in_=out_sb[b][:, :])


def _build(scales):
    key = (os.environ.get("BASS_SKIP_COLL", "0"), W8MODE,
           os.environ.get("BASS_PHASES", "9"),
           os.environ.get("BASS_DEBUG_DUMP", "0"), tuple(sorted(scales.items())))
    if key in _BUILD_CACHE:
        return _BUILD_CACHE[key]
    nc = bacc.Bacc("TRN2", target_bir_lowering=False, debug=False, num_devices=N_CORES)
    io = {
        "qa": nc.dram_tensor("qa", [128, NCH, 2, 128], BF16, kind="ExternalInput").ap(),
        "ka": nc.dram_tensor("ka", [128, NCH, 2, 128], BF16, kind="ExternalInput").ap(),
        "va": nc.dram_tensor("va", [128, NCH, 2, 128], BF16, kind="ExternalInput").ap(),
        "veff": nc.dram_tensor("veff", [BPC, C, P], BF16, kind="ExternalInput").ap(),
        "wq": _wtensor(nc, "wq", [128, NCH, 4, 128]),
        "wk": _wtensor(nc, "wk", [128, NCH, 4, 128]),
        "wv": _wtensor(nc, "wv", [128, NCH, 512]),
        "wfc": _wtensor(nc, "wfc", [128, NPT, 4, 512]),
        "bng": nc.dram_tensor("bng", [C, 1], F32, kind="ExternalInput").ap(),
        "bnb": nc.dram_tensor("bnb", [C, 1], F32, kind="ExternalInput").ap(),
        "out": nc.dram_tensor("out", [BPC, C, P], BF16, kind="ExternalOutput").ap(),
    }
    if os.environ.get("BASS_DEBUG_DUMP", "0") == "1":
        io.update({
            "dbg_qt": nc.dram_tensor("dbg_qt", [BPC, 128, 512], BF16, kind="ExternalOutput").ap(),
            "dbg_kt": nc.dram_tensor("dbg_kt", [BPC, 128, 512], BF16, kind="ExternalOutput").ap(),
            "dbg_v": nc.dram_tensor("dbg_v", [BPC, 128, 512], BF16, kind="ExternalOutput").ap(),
            "dbg_osc": nc.dram_tensor("dbg_osc", [BPC, 128, 512], F32, kind="ExternalOutput").ap(),
            "dbg_xhat": nc.dram_tensor("dbg_xhat", [BPC, 128, 512], BF16, kind="ExternalOutput").ap(),
        })
    from contextlib import ExitStack
    with tile.TileContext(nc) as tc, ExitStack() as ctx:
        _emit(ctx, nc, tc, io, scales)
    nc.compile()
    _BUILD_CACHE[key] = nc
    return nc


def _pow2_scale(w):
    m = float(np.abs(w).max())
    return float(2.0 ** np.floor(np.log2(FP8_MAX_TARGET / m)))


def _wtensor(nc, name, shape):
    if W8MODE == "bf16":
        return nc.dram_tensor(name, shape, mybir.dt.bfloat16,
                              kind="ExternalInput").ap()
    return nc.dram_tensor(name, shape, mybir.dt.uint8,
                          kind="ExternalInput").bitcast(FP8).ap()


def _q8(w, s):
    w = np.asarray(w, np.float32) * s
    if W8MODE == "bf16":
        return np.ascontiguousarray(w.astype(ml_dtypes.bfloat16))
    dt8 = ml_dtypes.float8_e3m4 if W8MODE == "e3" else ml_dtypes.float8_e4m3
    return np.ascontiguousarray(w.astype(dt8)).view(np.uint8)


def _bf16(x):
    return np.ascontiguousarray(np.asarray(x, np.float32).astype(ml_dtypes.bfloat16))


def _pack_acts(xf):
    # [b, c, p] f32 -> [128, NCH, b, c] bf16  (pixel-in-chunk, chunk, batch, channel)
    b = xf.shape[0]
    return _bf16(xf.transpose(2, 0, 1).reshape(NCH, 128, b, C).transpose(1, 0, 2, 3))


def kernel(v, k, q, w_qs, w_ks, w_vs, w_fc, ln_gamma, ln_beta, temperature,
           bn_gamma, bn_beta, **_ignored):
    v = np.asarray(v, np.float32)
    k = np.asarray(k, np.float32)
    q = np.asarray(q, np.float32)
    w_qs = np.asarray(w_qs, np.float32)
    w_ks = np.asarray(w_ks, np.float32)
    w_vs = np.asarray(w_vs, np.float32)
    w_fc = np.asarray(w_fc, np.float32)
    ln_gamma = np.asarray(ln_gamma, np.float32)
    ln_beta = np.asarray(ln_beta, np.float32)
    temp = float(np.asarray(temperature))
    bn_gamma = np.asarray(bn_gamma, np.float32)
    bn_beta = np.asarray(bn_beta, np.float32)

    qf = q.reshape(B, C, P)
    kf = k.reshape(B, C, P)
    vf = v.reshape(B, C, P)
    qa = _pack_acts(qf)   # [32, 128, 16, 128]
    ka = _pack_acts(kf)
    va = _pack_acts(vf)

    wqT = (w_qs / temp).T            # [P, D]
    wkT = w_ks.T
    wvT = w_vs.T
    wfcT_eff = (w_fc * ln_gamma[None, :]).T   # [D, P]
    s_q = _pow2_scale(wqT)
    s_k = _pow2_scale(wkT)
    s_v = _pow2_scale(wvT)
    s_fc = _pow2_scale(wfcT_eff)
    scales = {"s_q": s_q, "s_k": s_k, "s_v": s_v, "s_fc": s_fc}

    wq = _q8(wqT.reshape(NCH, 128, 4, 128).transpose(1, 0, 2, 3), s_q)
    wk = _q8(wkT.reshape(NCH, 128, 4, 128).transpose(1, 0, 2, 3), s_k)
    wv = _q8(wvT.reshape(NCH, 128, 512).transpose(1, 0, 2), s_v)
    wfc = _q8(wfcT_eff.reshape(4, 128, NPT, 512).transpose(1, 2, 0, 3), s_fc)
    bias_fc = (w_fc @ ln_beta).astype(np.float32)
    veff = vf + bias_fc[None, None, :]
    bng = np.ascontiguousarray(bn_gamma.reshape(C, 1))
    bnb = np.ascontiguousarray(bn_beta.reshape(C, 1))

    nc = _build(scales)
    in_maps = []
    for i in range(N_CORES):
        bs = slice(BPC * i, BPC * (i + 1))
        in_maps.append({
            "qa": np.ascontiguousarray(qa[:, :, bs, :]),
            "ka": np.ascontiguousarray(ka[:, :, bs, :]),
            "va": np.ascontiguousarray(va[:, :, bs, :]),
            # dims: [128 pixel-in-chunk, 32 chunk, 2 batch, 128 channel]
            "veff": _bf16(veff[bs]),
            "wq": wq, "wk": wk, "wv": wv, "wfc": wfc,
            "bng": bng, "bnb": bnb,
        })
    res = run_bass_kernel_spmd(nc, in_maps, core_ids=list(range(N_CORES)))
    global LAST_RESULTS
    LAST_RESULTS = res
    out = np.concatenate([np.asarray(res.results[i]["out"], dtype=np.float32)
                          for i in range(N_CORES)], axis=0)
    return out.reshape(B, C, HH, WW)


MODE = f"v2-{W8MODE}w-bf16a"


# revision 38
# speedup vs baseline: 1.3421x; 1.0854x over previous
"""Trainium2 Bass kernel for nn_MultiHeadAttention (channel-attention transformer block).

Math (per batch b, with X* = reshape(*, [C, P]), P = 4096, C = 128, D = 512):
  Q = Xq @ (Wq/temp)^T, K = Xk @ Wk^T, V = Xv @ Wv^T            [C, D]
  per head h (8 heads, ld=64): A_h = softmax(Q_h K_h^T); O_h = A_h V_h
  O = silu(O); O = (O - mean)/(unbiased_std + eps)   (LN affine folded into fc)
  out_pre = (v + Wfc@ln_beta) + O @ (Wfc*ln_gamma)^T
  out = BatchNorm2d(out_pre)   (batch stats over (b,h,w), biased var)

Sharding: data-parallel over batch, 2 batches per core on 8 cores; BatchNorm
statistics combined with a tiny AllReduce ([128,2] per core).

v2 design:
  - weights quantized to fp8 E3M4 with power-of-2 scales folded into existing
    per-row scalars (exp scale, sigmoid scale, LN-sqrt scale) => zero extra ops
  - activations/outputs in bf16 (DMA ~19MB/core vs 50MB f32)
  - Q/K projections computed weight-stationary so QT/KT land pre-transposed
  - attention ops batched across all 8 heads (1 exp, 1 reduce, 1 aT copy)
  - BN partial sums via accum_out on the residual-add + Square passes
  - bf16 output, upcast on host
"""

import os

import numpy as np
import ml_dtypes

import concourse.mybir as mybir
import concourse.tile as tile
from concourse import bacc
from concourse.bass_utils import run_bass_kernel_spmd
from concourse.masks import make_identity

# ---- problem constants (hardcoded per contract) ----
B, C, HH, WW = 16, 128, 64, 64
P = HH * WW           # 4096
NH, LD = 8, 64
D = NH * LD           # 512
N_CORES = 8
BPC = B // N_CORES    # 2 batches per core
NCH = P // 128        # 32 pixel chunks (contraction)
NPT = P // 512        # 8 output column tiles for fc
LN_EPS = 1e-6
BN_EPS = 1e-5
F32 = mybir.dt.float32
BF16 = mybir.dt.bfloat16
W8MODE = os.environ.get("BASS_W8", "e3")  # e3 | e4 | bf16
FP8 = {"e3": mybir.dt.float8e3, "e4": mybir.dt.float8e4,
       "bf16": mybir.dt.bfloat16}[W8MODE]
W8BYTES = 2 if W8MODE == "bf16" else 1
FP8_MAX_TARGET = {"e3": 14.0, "e4": 224.0, "bf16": 14.0}[W8MODE]

_BUILD_CACHE: dict = {}
LAST_RESULTS = None  # BassKernelResults of the most recent run (for profiling)

# host-side fp8 scales (power of two), computed at pack time, baked into build
_SCALES: dict = {}


def _emit(ctx, nc, tc, io, scales):
    PH = int(os.environ.get("BASS_PHASES", "9"))
    AF = mybir.ActivationFunctionType
    ALU = mybir.AluOpType
    AX = mybir.AxisListType
    s_q, s_k, s_v, s_fc = (scales[k] for k in ("s_q", "s_k", "s_v", "s_fc"))

    consts = ctx.enter_context(tc.tile_pool(name="consts", bufs=1))
    big = ctx.enter_context(tc.tile_pool(name="big", bufs=1))
    sb = ctx.enter_context(tc.tile_pool(name="sb", bufs=2))
    small = ctx.enter_context(tc.tile_pool(name="small", bufs=4))
    stat = ctx.enter_context(tc.tile_pool(name="stat", bufs=1))
    dram = ctx.enter_context(tc.tile_pool(name="dram", bufs=1, space="DRAM"))

    ident = consts.tile([128, 128], BF16, tag="ident", name="ident")
    identf = consts.tile([128, 128], F32, tag="identf", name="identf")
    make_identity(nc, identf)
    nc.vector.tensor_copy(out=ident, in_=identf)
    ones = consts.tile([128, 1], BF16, tag="ones", name="ones")
    nc.vector.memset(ones, 1.0)

    bng = consts.tile([128, 1], F32, tag="bng", name="bng")
    bnb = consts.tile([128, 1], F32, tag="bnb", name="bnb")
    epsbn = consts.tile([128, 1], F32, tag="epsbn", name="epsbn")
    nc.gpsimd.dma_start(out=bng, in_=io["bng"][:, :])
    nc.gpsimd.dma_start(out=bnb, in_=io["bnb"][:, :])
    nc.vector.memset(epsbn, BN_EPS)

    # ---- prefetch: everything lives in SBUF (quartered DMAs so compute can
    # start as soon as the first chunks land)
    qa_sb = big.tile([128, NCH, 2, 128], BF16, tag="qa_sb", name="qa_sb")
    ka_sb = big.tile([128, NCH, 2, 128], BF16, tag="ka_sb", name="ka_sb")
    va_sb = big.tile([128, NCH, 2, 128], BF16, tag="va_sb", name="va_sb")
    wq_sb = big.tile([128, NCH, 4, 128], FP8, tag="wq_sb", name="wq_sb")
    wk_sb = big.tile([128, NCH, 4, 128], FP8, tag="wk_sb", name="wk_sb")
    wv_sb = big.tile([128, NCH, 512], FP8, tag="wv_sb", name="wv_sb")
    QTR = NCH // 4
    for q4 in range(4):
        cs = slice(QTR * q4, QTR * (q4 + 1))
        nc.sync.dma_start(out=qa_sb[:, cs, :, :], in_=io["qa"][q4])
        nc.scalar.dma_start(out=wq_sb[:, cs, :, :], in_=io["wq"][q4])
    for q4 in range(4):
        cs = slice(QTR * q4, QTR * (q4 + 1))
        nc.sync.dma_start(out=ka_sb[:, cs, :, :], in_=io["ka"][q4])
        nc.scalar.dma_start(out=wk_sb[:, cs, :, :], in_=io["wk"][q4])
    for q4 in range(4):
        cs = slice(QTR * q4, QTR * (q4 + 1))
        nc.sync.dma_start(out=va_sb[:, cs, :, :], in_=io["va"][q4])
        nc.scalar.dma_start(out=wv_sb[:, cs, :], in_=io["wv"][q4])

    # residual (+ folded fc bias) and fc weights on the gpsimd queue
    veff_sb = []
    for b in range(BPC):
        t = big.tile([128, P], BF16, tag=f"veff{b}", name=f"veff{b}")
        nc.gpsimd.dma_start(out=t, in_=io["veff"][b, :, :])
        veff_sb.append(t)
    wfc_sb = big.tile([128, NPT, 4, 512], FP8, tag="wfc_sb", name="wfc_sb")
    nc.gpsimd.dma_start(out=wfc_sb, in_=io["wfc"])
    out_sb = [big.tile([128, P], BF16, tag=f"outb{b}", name=f"outb{b}")
              for b in range(BPC)]

    # ---- phase A: QKV projections, accumulating over the P=4096 contraction.
    # Q,K weight-stationary with both batches as one 256-wide moving operand
    # (outputs arrive transposed [d, b|c]); V activation-stationary 512-wide.
    ctx_a1 = tc.tile_pool(name="ps_qk", bufs=3, space="PSUM")
    ps_qk = ctx_a1.__enter__()
    ctx_a2 = tc.tile_pool(name="ps_v", bufs=1, space="PSUM")
    ps_v = ctx_a2.__enter__()
    warm = ps_v.tile([128, 128], BF16, tag="warm", name="warm")
    nc.tensor.transpose(warm[:, :], ident[:, :], ident[:, :])

    qkv_sb = []
    for b in range(BPC):
        QT_sb = sb.tile([128, 512], BF16, tag=f"QT_sb{b}", name=f"QT_sb{b}")
        KT_sb = sb.tile([128, 512], BF16, tag=f"KT_sb{b}", name=f"KT_sb{b}")
        V_sb = sb.tile([128, 512], BF16, tag=f"V_sb{b}", name=f"V_sb{b}")
        qkv_sb.append((QT_sb, KT_sb, V_sb))

    if PH >= 2:
        for w_sb, a_sb, which in ((wq_sb, qa_sb, 0), (wk_sb, ka_sb, 1)):
            for dc in range(4):
                pq = ps_qk.tile([128, 256], F32, tag="qkp", name="qkp")
                for chunk in range(NCH):
                    nc.tensor.matmul(pq[:, :], w_sb[:, chunk, dc, :],
                                     a_sb[:, chunk, :, :],
                                     start=chunk == 0, stop=chunk == NCH - 1)
                fo = dc * 128
                dst0 = qkv_sb[0][which]
                dst1 = qkv_sb[1][which]
                nc.vector.tensor_copy(out=dst0[:, fo:fo + 128], in_=pq[:, 0:128])
                nc.vector.tensor_copy(out=dst1[:, fo:fo + 128], in_=pq[:, 128:256])
        for b in range(BPC):
            vp = ps_v.tile([128, 512], F32, tag=f"vp{b}", name=f"vp{b}")
            for chunk in range(NCH):
                nc.tensor.matmul(vp[:, :], va_sb[:, chunk, b, :],
                                 wv_sb[:, chunk, :],
                                 start=chunk == 0, stop=chunk == NCH - 1)
            if b == 0:
                nc.vector.tensor_copy(out=qkv_sb[b][2], in_=vp[:, :])
            else:
                nc.scalar.copy(out=qkv_sb[b][2], in_=vp[:, :])
    ctx_a2.__exit__(None, None, None)
    ctx_a1.__exit__(None, None, None)

    ps_s = ctx.enter_context(tc.tile_pool(name="ps_s", bufs=2, space="PSUM"))
    ps_xt = ctx.enter_context(tc.tile_pool(name="ps_xt", bufs=1, space="PSUM"))
    ps_o = ctx.enter_context(tc.tile_pool(name="ps_o", bufs=1, space="PSUM"))
    ps_fc = ctx.enter_context(tc.tile_pool(name="ps_fc", bufs=2, space="PSUM"))

    # per-channel partial sums: cols 0..15 = sum(out) per (b,pt), 16..31 = sum(out^2)
    pcols = stat.tile([128, 32], F32, tag="pcols", name="pcols")

    exp_scale = 1.0 / (s_q * s_k)
    sig_scale = 1.0 / s_v
    sqrt_scale = (float(D) / (D - 1)) * s_fc * s_fc
    eps_s = LN_EPS * s_v * s_fc

    dbg = os.environ.get("BASS_DEBUG_DUMP", "0") == "1" and "dbg_qt" in io
    if dbg:
        for b in range(BPC):
            nc.gpsimd.dma_start(out=io["dbg_qt"][b], in_=qkv_sb[b][0][:, :])
            nc.gpsimd.dma_start(out=io["dbg_kt"][b], in_=qkv_sb[b][1][:, :])
            nc.gpsimd.dma_start(out=io["dbg_v"][b], in_=qkv_sb[b][2][:, :])

    if PH < 3:
        for b in range(BPC):
            nc.vector.memset(out_sb[b], 0.0)
            eng = nc.sync if b == 0 else nc.scalar
            eng.dma_start(out=io["out"][b, :, :], in_=out_sb[b][:, :])
        return

    # ---- phase B: attention, ST formulation. ST_h = K_h Q_h^T comes out
    # [e, c]; exp(ST) is A^T which is exactly the AV stationary operand, so no
    # PE transposes or PSUM->SBUF shuffles are needed. Softmax denominators
    # come from a 1-wide matmul against a ones vector (same stationary).
    # All exps are contiguous so the ACT Exp table loads once.
    ls = ps_o.tile([128, 16], F32, tag="ls", name="ls")
    eftp = ctx.enter_context(tc.tile_pool(name="eftp", bufs=16))
    Ops = []
    efts = {}
    for b in range(BPC):
        QT_sb, KT_sb, V_sb = qkv_sb[b]
        for h in range(NH):
            po = (h % 2) * 64
            fo = (h // 2) * 128
            ST = ps_s.tile([128, 128], F32, tag="S", name="S")
            nc.tensor.matmul(ST[:, :], KT_sb[po:po + 64, fo:fo + 128],
                             QT_sb[po:po + 64, fo:fo + 128], start=True, stop=True)
            eft = eftp.tile([128, 128], BF16, tag="eft", name="eft")
            nc.scalar.activation(out=eft, in_=ST[:, :], func=AF.Exp,
                                 scale=exp_scale)
            efts[(b, h)] = eft
    Oscs = []
    for b in range(BPC):
        V_sb = qkv_sb[b][2]
        Opsum = ps_o.tile([128, 512], F32, tag=f"O{b}", name=f"O{b}")
        Ops.append(Opsum)
        for h in range(NH):
            nc.tensor.matmul(Opsum[:, h * 64:(h + 1) * 64], efts[(b, h)][:, :],
                             V_sb[:, h * 64:(h + 1) * 64], start=True, stop=True)
            nc.tensor.matmul(ls[:, b * 8 + h:b * 8 + h + 1], efts[(b, h)][:, :],
                             ones[:, :], start=True, stop=True)

    for b in range(BPC):
        rs = small.tile([128, 8], F32, tag="rs", name="rs")
        nc.vector.reciprocal(rs, ls[:, b * 8:b * 8 + 8])
        Osc = sb.tile([128, 512], F32, tag=f"Osc{b}", name=f"Osc{b}")
        for h in range(NH):
            nc.vector.tensor_scalar_mul(out=Osc[:, h * 64:(h + 1) * 64],
                                        in0=Ops[b][:, h * 64:(h + 1) * 64],
                                        scalar1=rs[:, h:h + 1])
        Oscs.append(Osc)
        if dbg:
            nc.gpsimd.dma_start(out=io["dbg_osc"][b], in_=Osc[:, :])

    if PH < 4:
        for b in range(BPC):
            nc.vector.memset(out_sb[b], 0.0)
            eng = nc.sync if b == 0 else nc.scalar
            eng.dma_start(out=io["out"][b, :, :], in_=out_sb[b][:, :])
        return

    # ---- phase C: silu + LN (sigmoids grouped, then sqrts, for table reuse)
    Osws = []
    mvs = []
    for b in range(BPC):
        sg = sb.tile([128, D], F32, tag=f"sg{b}", name=f"sg{b}")
        nc.scalar.activation(out=sg, in_=Oscs[b], func=AF.Sigmoid, scale=sig_scale)
        Osw = sb.tile([128, D], F32, tag=f"Osw{b}", name=f"Osw{b}")
        nc.vector.tensor_mul(out=Osw, in0=Oscs[b], in1=sg)
        st6 = small.tile([128, 6], F32, tag="st6", name="st6")
        nc.vector.bn_stats(out=st6, in_=Osw)
        mv = small.tile([128, 2], F32, tag=f"mv{b}", name=f"mv{b}")
        nc.vector.bn_aggr(out=mv, in_=st6)
        Osws.append(Osw)
        mvs.append(mv)
    xTs = []
    for b in range(BPC):
        sd = small.tile([128, 1], F32, tag="sd", name="sd")
        nc.scalar.activation(out=sd, in_=mvs[b][:, 1:2], func=AF.Sqrt,
                             scale=sqrt_scale)
        nc.vector.tensor_scalar_add(out=sd, in0=sd, scalar1=eps_s)
        rstd = small.tile([128, 1], F32, tag="rstd", name="rstd")
        nc.vector.reciprocal(rstd, sd)
        xhat = sb.tile([128, D], BF16, tag=f"xhat{b}", name=f"xhat{b}")
        nc.vector.tensor_scalar(out=xhat, in0=Osws[b], scalar1=mvs[b][:, 0:1],
                                scalar2=rstd, op0=ALU.subtract, op1=ALU.mult)
        xT = sb.tile([128, 4, 128], BF16, tag=f"xT{b}", name=f"xT{b}")
        for dc in range(4):
            tp = ps_xt.tile([128, 128], BF16, tag="xtp", name="xtp")
            nc.tensor.transpose(tp[:, :], xhat[:, dc * 128:(dc + 1) * 128],
                                ident[:, :])
            if dc % 2 == 0:
                nc.vector.tensor_copy(out=xT[:, dc, :], in_=tp[:, :])
            else:
                nc.scalar.copy(out=xT[:, dc, :], in_=tp[:, :])
        xTs.append(xT)
        if dbg:
            nc.gpsimd.dma_start(out=io["dbg_xhat"][b], in_=xhat[:, :])

    if PH < 5:
        for b in range(BPC):
            nc.vector.memset(out_sb[b], 0.0)
            eng = nc.sync if b == 0 else nc.scalar
            eng.dma_start(out=io["out"][b, :, :], in_=out_sb[b][:, :])
        return

    # ---- phase D: fc + residual + BN partial sums
    junk = sb.tile([128, 512], BF16, tag="junk", name="junk")
    for b in range(BPC):
        for pt in range(NPT):
            O2 = ps_fc.tile([128, 512], F32, tag="O2", name="O2")
            for dc in range(4):
                nc.tensor.matmul(O2[:, :], xTs[b][:, dc, :], wfc_sb[:, pt, dc, :],
                                 start=dc == 0, stop=dc == 3)
            seg = out_sb[b][:, pt * 512:(pt + 1) * 512]
            i = b * NPT + pt
            nc.vector.scalar_tensor_tensor(
                out=seg, in0=O2[:, :], scalar=1.0,
                in1=veff_sb[b][:, pt * 512:(pt + 1) * 512],
                op0=ALU.mult, op1=ALU.add, accum_out=pcols[:, i:i + 1])
            nc.scalar.activation(out=junk, in_=seg, func=AF.Square,
                                 accum_out=pcols[:, 16 + i:17 + i])

    # ---- phase E: BN stats AllReduce + normalize + store
    stats2 = stat.tile([128, 2], F32, tag="stats2", name="stats2")
    nc.vector.reduce_sum(stats2[:, 0:1], pcols[:, 0:16], axis=AX.X)
    nc.vector.reduce_sum(stats2[:, 1:2], pcols[:, 16:32], axis=AX.X)

    cin = dram.tile([128, 2], F32, tag="cin", name="cin")
    cout = dram.tile([128, 2], F32, tag="cout", name="cout")
    nc.gpsimd.dma_start(out=cin[:, :], in_=stats2)
    if os.environ.get("BASS_SKIP_COLL", "0") == "1":
        nc.gpsimd.dma_start(out=cout[:, :], in_=cin[:, :])
    else:
        nc.gpsimd.collective_compute(
            "AllReduce",
            mybir.AluOpType.add,
            replica_groups=[list(range(N_CORES))],
            ins=[cin.opt()],
            outs=[cout.opt()],
        )
    red = stat.tile([128, 2], F32, tag="red", name="red")
    nc.gpsimd.dma_start(out=red[:, :], in_=cout[:, :])

    inv_n = 1.0 / float(B * P)
    mean = small.tile([128, 1], F32, tag="mean", name="mean")
    nc.vector.tensor_scalar_mul(out=mean, in0=red[:, 0:1], scalar1=inv_n)
    ex2 = small.tile([128, 1], F32, tag="ex2", name="ex2")
    nc.vector.tensor_scalar_mul(out=ex2, in0=red[:, 1:2], scalar1=inv_n)
    msq = small.tile([128, 1], F32, tag="msq", name="msq")
    nc.vector.tensor_mul(out=msq, in0=mean, in1=mean)
    var = small.tile([128, 1], F32, tag="var", name="var")
    nc.vector.tensor_sub(out=var, in0=ex2, in1=msq)
    sdv = small.tile([128, 1], F32, tag="sdv", name="sdv")
    nc.scalar.activation(out=sdv, in_=var, func=AF.Sqrt, bias=epsbn)
    invs = small.tile([128, 1], F32, tag="invs", name="invs")
    nc.vector.reciprocal(invs, sdv)
    scl = small.tile([128, 1], F32, tag="scl", name="scl")
    nc.vector.tensor_mul(out=scl, in0=bng, in1=invs)
    tmp = small.tile([128, 1], F32, tag="tmp", name="tmp")
    nc.vector.tensor_mul(out=tmp, in0=mean, in1=scl)
    shf = small.tile([128, 1], F32, tag="shf", name="shf")
    nc.vector.tensor_sub(out=shf, in0=bnb, in1=tmp)

    for b in range(BPC):
        nc.vector.tensor_scalar(out=out_sb[b][:, :], in0=out_sb[b][:, :],
                                scalar1=scl, scalar2=shf,
                                op0=ALU.mult, op1=ALU.add)
        eng = nc.sync if b == 0 else nc.scalar
        eng.dma_start(out=io["out"][b, :, :], in_=out_sb[b][:, :])


def _build(scales):
    key = (os.environ.get("BASS_SKIP_COLL", "0"), W8MODE,
           os.environ.get("BASS_PHASES", "9"),
           os.environ.get("BASS_DEBUG_DUMP", "0"), tuple(sorted(scales.items())))
    if key in _BUILD_CACHE:
        return _BUILD_CACHE[key]
    nc = bacc.Bacc("TRN2", target_bir_lowering=False, debug=False, num_devices=N_CORES)
    io = {
        "qa": nc.dram_tensor("qa", [4, 128, NCH // 4, 2, 128], BF16, kind="ExternalInput").ap(),
        "ka": nc.dram_tensor("ka", [4, 128, NCH // 4, 2, 128], BF16, kind="ExternalInput").ap(),
        "va": nc.dram_tensor("va", [4, 128, NCH // 4, 2, 128], BF16, kind="ExternalInput").ap(),
        "veff": nc.dram_tensor("veff", [BPC, C, P], BF16, kind="ExternalInput").ap(),
        "wq": _wtensor(nc, "wq", [4, 128, NCH // 4, 4, 128]),
        "wk": _wtensor(nc, "wk", [4, 128, NCH // 4, 4, 128]),
        "wv": _wtensor(nc, "wv", [4, 128, NCH // 4, 512]),
        "wfc": _wtensor(nc, "wfc", [128, NPT, 4, 512]),
        "bng": nc.dram_tensor("bng", [C, 1], F32, kind="ExternalInput").ap(),
        "bnb": nc.dram_tensor("bnb", [C, 1], F32, kind="ExternalInput").ap(),
        "out": nc.dram_tensor("out", [BPC, C, P], BF16, kind="ExternalOutput").ap(),
    }
    if os.environ.get("BASS_DEBUG_DUMP", "0") == "1":
        io.update({
            "dbg_qt": nc.dram_tensor("dbg_qt", [BPC, 128, 512], BF16, kind="ExternalOutput").ap(),
            "dbg_kt": nc.dram_tensor("dbg_kt", [BPC, 128, 512], BF16, kind="ExternalOutput").ap(),
            "dbg_v": nc.dram_tensor("dbg_v", [BPC, 128, 512], BF16, kind="ExternalOutput").ap(),
            "dbg_osc": nc.dram_tensor("dbg_osc", [BPC, 128, 512], F32, kind="ExternalOutput").ap(),
            "dbg_xhat": nc.dram_tensor("dbg_xhat", [BPC, 128, 512], BF16, kind="ExternalOutput").ap(),
        })
    from contextlib import ExitStack
    with tile.TileContext(nc) as tc, ExitStack() as ctx:
        _emit(ctx, nc, tc, io, scales)
    nc.compile()
    _BUILD_CACHE[key] = nc
    return nc


def _pow2_scale(w):
    m = float(np.abs(w).max())
    return float(2.0 ** np.floor(np.log2(FP8_MAX_TARGET / m)))


def _wtensor(nc, name, shape):
    if W8MODE == "bf16":
        return nc.dram_tensor(name, shape, mybir.dt.bfloat16,
                              kind="ExternalInput").ap()
    return nc.dram_tensor(name, shape, mybir.dt.uint8,
                          kind="ExternalInput").bitcast(FP8).ap()


def _q8(w, s):
    w = np.asarray(w, np.float32) * s
    if W8MODE == "bf16":
        return np.ascontiguousarray(w.astype(ml_dtypes.bfloat16))
    dt8 = ml_dtypes.float8_e3m4 if W8MODE == "e3" else ml_dtypes.float8_e4m3
    return np.ascontiguousarray(w.astype(dt8)).view(np.uint8)


def _bf16(x):
    return np.ascontiguousarray(np.asarray(x, np.float32).astype(ml_dtypes.bfloat16))


def _pack_acts(xf):
    # [b, c, p] f32 -> [128, NCH, b, c] bf16  (pixel-in-chunk, chunk, batch, channel)
    b = xf.shape[0]
    return _bf16(xf.transpose(2, 0, 1).reshape(NCH, 128, b, C).transpose(1, 0, 2, 3))


def _qmajor(x):
    # [128, NCH, ...] -> [4, 128, NCH//4, ...] (contiguous per-quarter DMA)
    s = x.shape
    return np.ascontiguousarray(
        x.reshape(128, 4, NCH // 4, *s[2:]).transpose(1, 0, 2, *range(3, x.ndim + 1)))


def kernel(v, k, q, w_qs, w_ks, w_vs, w_fc, ln_gamma, ln_beta, temperature,
           bn_gamma, bn_beta, **_ignored):
    v = np.asarray(v, np.float32)
    k = np.asarray(k, np.float32)
    q = np.asarray(q, np.float32)
    w_qs = np.asarray(w_qs, np.float32)
    w_ks = np.asarray(w_ks, np.float32)
    w_vs = np.asarray(w_vs, np.float32)
    w_fc = np.asarray(w_fc, np.float32)
    ln_gamma = np.asarray(ln_gamma, np.float32)
    ln_beta = np.asarray(ln_beta, np.float32)
    temp = float(np.asarray(temperature))
    bn_gamma = np.asarray(bn_gamma, np.float32)
    bn_beta = np.asarray(bn_beta, np.float32)

    qf = q.reshape(B, C, P)
    kf = k.reshape(B, C, P)
    vf = v.reshape(B, C, P)
    qa = _pack_acts(qf)   # [32, 128, 16, 128]
    ka = _pack_acts(kf)
    va = _pack_acts(vf)

    wqT = (w_qs / temp).T            # [P, D]
    wkT = w_ks.T
    wvT = w_vs.T
    wfcT_eff = (w_fc * ln_gamma[None, :]).T   # [D, P]
    s_q = _pow2_scale(wqT)
    s_k = _pow2_scale(wkT)
    s_v = _pow2_scale(wvT)
    s_fc = _pow2_scale(wfcT_eff)
    scales = {"s_q": s_q, "s_k": s_k, "s_v": s_v, "s_fc": s_fc}

    wq = _qmajor(_q8(wqT.reshape(NCH, 128, 4, 128).transpose(1, 0, 2, 3), s_q))
    wk = _qmajor(_q8(wkT.reshape(NCH, 128, 4, 128).transpose(1, 0, 2, 3), s_k))
    wv = _qmajor(_q8(wvT.reshape(NCH, 128, 512).transpose(1, 0, 2), s_v))
    wfc = _q8(wfcT_eff.reshape(4, 128, NPT, 512).transpose(1, 2, 0, 3), s_fc)
    bias_fc = (w_fc @ ln_beta).astype(np.float32)
    veff = vf + bias_fc[None, None, :]
    bng = np.ascontiguousarray(bn_gamma.reshape(C, 1))
    bnb = np.ascontiguousarray(bn_beta.reshape(C, 1))

    nc = _build(scales)
    in_maps = []
    for i in range(N_CORES):
        bs = slice(BPC * i, BPC * (i + 1))
        in_maps.append({
            "qa": _qmajor(qa[:, :, bs, :]),
            "ka": _qmajor(ka[:, :, bs, :]),
            "va": _qmajor(va[:, :, bs, :]),
            "veff": _bf16(veff[bs]),
            "wq": wq, "wk": wk, "wv": wv, "wfc": wfc,
            "bng": bng, "bnb": bnb,
        })
    res = run_bass_kernel_spmd(nc, in_maps, core_ids=list(range(N_CORES)))
    global LAST_RESULTS
    LAST_RESULTS = res
    out = np.concatenate([np.asarray(res.results[i]["out"], dtype=np.float32)
                          for i in range(N_CORES)], axis=0)
    return out.reshape(B, C, HH, WW)


MODE = f"v2-{W8MODE}w-bf16a"
